# revision 33
# baseline (speedup 1.0000x reference)
"""Trainium2 Bass kernel for causal MultiHeadAttention (B=4,S=2048,E=1024,H=16).

Sharding: 8 cores = (batch b, head-half) grid. Core c handles batch c//2 and
heads [8*(c%2), 8*(c%2)+8). Each core computes its 8 heads' attention and the
partial output projection (its 512 rows of Wo); the host sums the two fp16
partials per batch and adds the bias. ~280us/core vs the 382us v1 baseline.

Design notes (what made it fast):
  - Every matmul is a 128-partition FWL-eligible weight: scores use the
    stacked head-pair K as the stationary operand ([128 = h0 dh | h1 dh])
    against a per-head zero-padded Q moving operand (the zero half kills
    the cross-head term), so the PE runs warm 216ns/512-col MMs
    back-to-back with weight loads fully hidden. (PE row/col tiling and
    fp8 DoubleRow were measured dead ends on this hardware: non-FWL
    weight loads serialize ~100ns/MM and DoubleRow double-pumping does
    not engage, while fp8 probs cost 8x in accuracy.)
  - PV: bf16 [128,128] V weights laid out [V | ones | zeros]; one matmul
    per (head, t-tile) accumulated over the unit, partial-N on diagonal
    tiles. The ones column makes psum row 64 the softmax denominator.
  - Causality at t-tile granularity everywhere (only 62.5% of score/PV
    work is computed); the 4 diagonal 128x128 subtiles per (head, chunk)
    are fixed post-exp with one 0/1 bf16 multiply on the idle GpSimd.
  - ACT exp is the P3 pacer (~158us): emission interleaves whole P2
    projection chains and per-chunk P4 output-projection chains between
    score groups so the PE always has FWL work while ACT chews exps;
    psum pools pace scores one pr ahead of exp (PSUM: 4 banks scores,
    2 PV, 2 projections).
  - Few, large DMAs: one multi-dim descriptor per x chunk / weight pane
    (a dma_start costs ~600ns of sequencer time, so 97 startup triggers
    were ~58us of serialized pacing in earlier versions); fp16 output
    streams out per chunk; denominator reciprocal via
    reciprocal_approx_fast and a 4-trigger DRAM-bounce broadcast.
  - Warm-up matmuls during the DMA-paced first chunk keep the PE p-state
    high (cold-start MMs run 2x slow otherwise).
"""

import sys

if "/opt/trn_rl_repo" not in sys.path:
    sys.path.insert(0, "/opt/trn_rl_repo")

import numpy as np
from collections import deque
from contextlib import ExitStack

B, S, E, H = 4, 2048, 1024, 16
DH = E // H          # 64
NCORES = 8
NH = 8               # local heads per core
HP = NH // 2         # head pairs
P = 128
NE = E // P          # 8 e-tiles
NT = S // P          # 16 t-tiles
CH = 512
NCH = S // CH        # 4 q-chunks
SCALE = 1.0 / 8.0    # 1/sqrt(DH)

_CACHE = {}


def _build_nc():
    import concourse.mybir as mybir
    import concourse.tile as tile
    import concourse.bass as bass
    from concourse import bacc

    f32 = mybir.dt.float32
    f16 = mybir.dt.float16
    bf16 = mybir.dt.bfloat16
    Exp = mybir.ActivationFunctionType.Exp
    PSUM = bass.MemorySpace.PSUM

    nc = bacc.Bacc(None)
    x_d = nc.dram_tensor("x", [E, S], bf16, kind="ExternalInput")  # pre-transposed
    wq_d = nc.dram_tensor("wq", [E, NH * DH], bf16, kind="ExternalInput")
    wk_d = nc.dram_tensor("wk", [E, NH * DH], bf16, kind="ExternalInput")
    wv_d = nc.dram_tensor("wv", [E, NH * DH], bf16, kind="ExternalInput")
    wo_d = nc.dram_tensor("wo", [NH * DH, E], bf16, kind="ExternalInput")
    msk_d = nc.dram_tensor("mask", [P, P], bf16, kind="ExternalInput")
    out_d = nc.dram_tensor("out", [S, E], f16, kind="ExternalOutput")

    with ExitStack() as ctx:
        tc = ctx.enter_context(tile.TileContext(nc))
        persist = ctx.enter_context(tc.tile_pool(name="persist", bufs=1))

        qp = persist.tile([P, NH, S], bf16)            # per-head, half zero
        ks = persist.tile([P, HP, S], bf16)            # rows = stacked pair dh
        vf = persist.tile([P, NT, NH, P], bf16)        # V | ones | zeros
        msk = persist.tile([P, P], bf16)               # 0/1 causal subtile
        outTs = [persist.tile([P, S], bf16, tag=f"outT{i}", name="outT")
                 for i in range(HP)]

        # zero fills first (both engine queues are empty at t0)
        # vf ones column and zero padding (replaces v1's 4MB zz DMA)
        nc.vector.memset(vf[:, :, :, DH:DH + 1], 1.0)
        nc.vector.memset(vf[:, :, :, DH + 1:P], 0.0)
        # qp: the half of each head's 128 rows not holding Q stays zero so
        # the K=128 stacked-K score matmul drops the other head's term
        nc.gpsimd.memset(qp[DH:P, 0::2, :], 0.0)
        nc.gpsimd.memset(qp[0:DH, 1::2, :], 0.0)

        # ---- input DMAs, critical-path order ----
        # sync queue: x chunk 0 first, then the weights that feed P2
        xtp = ctx.enter_context(tc.tile_pool(name="xtp", bufs=1))
        xcs = {}

        def emit_xt(c, split=1):
            xc = xtp.tile([P, NE, CH], bf16, tag=f"xt{c % 2}", name="xt")
            step = NE // split
            for k in range(split):
                src_ap = bass.AP(
                    tensor=x_d.tensor if hasattr(x_d, 'tensor') else x_d,
                    offset=c * CH + k * step * P * S,
                    ap=[[S, P], [P * S, step], [1, CH]])
                nc.sync.dma_start(out=xc[:, k * step:(k + 1) * step, :],
                                  in_=src_ap)
            xcs[c] = xc

        emit_xt(0, split=4)
        wvall = persist.tile([P, NE, NH * DH], bf16, tag="wvall", name="wvall")
        nc.sync.dma_start(
            out=wvall,
            in_=bass.AP(
                tensor=wv_d.tensor if hasattr(wv_d, 'tensor') else wv_d,
                offset=0,
                ap=[[NH * DH, P], [P * NH * DH, NE], [1, NH * DH]]))
        emit_xt(1)
        wt2s = {}

        def emit_wt2():
            for ech in range(E // CH):
                for hp in range(HP):
                    w2 = persist.tile([P, CH], bf16, tag=f"wt2{ech}_{hp}",
                                      name="w2")
                    nc.sync.dma_start(
                        out=w2,
                        in_=wo_d[hp * P:(hp + 1) * P,
                                 ech * CH:(ech + 1) * CH])
                    wt2s[(ech, hp)] = w2

        # scalar queue (idle after startup): wq/wk per head pair + mask
        nc.scalar.dma_start(out=msk, in_=msk_d[:])
        wts = {}
        for hp in range(HP):
            for wi, wd in enumerate((wq_d, wk_d)):
                wt = persist.tile([P, NE, P], bf16, tag=f"wt{hp}{wi}",
                                  name="wt")
                nc.scalar.dma_start(
                    out=wt,
                    in_=bass.AP(
                        tensor=wd.tensor if hasattr(wd, 'tensor') else wd,
                        offset=hp * P,
                        ap=[[NH * DH, P], [P * NH * DH, NE], [1, P]]))
                wts[(hp, wi)] = wt

        # ---- pools ----
        prp = ctx.enter_context(tc.tile_pool(name="prp", bufs=2, space=PSUM))
        scp = ctx.enter_context(tc.tile_pool(name="scp", bufs=2, space=PSUM))
        pvp = ctx.enter_context(tc.tile_pool(name="pvp", bufs=2, space=PSUM))
        ptp = ctx.enter_context(tc.tile_pool(name="ptp", bufs=18))
        pop = ctx.enter_context(tc.tile_pool(name="pop", bufs=6))
        dnp = ctx.enter_context(tc.tile_pool(name="dnp", bufs=3))
        bcp = ctx.enter_context(tc.tile_pool(name="bcp", bufs=2))
        osb = ctx.enter_context(tc.tile_pool(name="osb", bufs=3))
        drp = ctx.enter_context(tc.tile_pool(name="drp", bufs=2, space="DRAM"))

        # dummy warm-up matmuls (write an unread scp column; weights = msk,
        # moving = a memset tile): keep the PE pipeline busy through the
        # DMA-paced startup so P2 chunk 0 runs at full clock
        wrm = persist.tile([P, CH], bf16, tag="wrm", name="wrm")
        nc.vector.memset(wrm, 0.0)

        def emit_warm(n):
            ws = scp.tile([P, 2, CH], f32, tag="sp", name="sp")
            for _ in range(n):
                nc.tensor.matmul(ws[:, 0, :], msk, wrm, start=True, stop=True)

        # ---- filler work items (one whole PE chain each, ~1.8us) ----
        def gen_p2b(c, hp, wi):
            ps = prp.tile([P, CH], f32, tag="prj", name="prj")
            wt = wts[(hp, wi)]
            for et in range(NE):
                nc.tensor.matmul(ps, wt[:, et, :], xcs[c][:, et, :],
                                 start=(et == 0), stop=(et == NE - 1),
                                 skip_group_check=True)
            cs = slice(c * CH, (c + 1) * CH)
            if wi == 0:
                nc.vector.tensor_copy(
                    out=qp[0:DH, 2 * hp, cs], in_=ps[0:DH, :])
                nc.vector.tensor_copy(
                    out=qp[DH:P, 2 * hp + 1, cs], in_=ps[DH:P, :])
            else:
                nc.vector.tensor_copy(out=ks[:, hp, cs], in_=ps)
            yield

        def gen_p2a(c, sti):
            st = 4 * c + sti
            ps = prp.tile([P, CH], f32, tag="prj", name="prj")
            for et in range(NE):
                nc.tensor.matmul(
                    ps, xcs[c][:, et, sti * P:(sti + 1) * P], wvall[:, et, :],
                    start=(et == 0), stop=(et == NE - 1),
                    skip_group_check=True)
            nc.vector.tensor_copy(
                out=vf[:, st, :, 0:DH],
                in_=ps.rearrange("p (h d) -> p h d", h=NH))
            yield

        def gen_p4(c, ech, sti):
            st = 4 * c + sti
            ps = prp.tile([P, CH], f32, tag="prj", name="prj")
            for hp in range(HP):
                nc.tensor.matmul(
                    ps, outTs[hp][:, st * P:(st + 1) * P], wt2s[(ech, hp)],
                    start=(hp == 0), stop=(hp == HP - 1),
                    skip_group_check=True)
            ob = osb.tile([P, CH], f16, tag="ob", name="ob")
            nc.vector.tensor_copy(out=ob, in_=ps)
            q = nc.scalar if c == NCH - 1 else nc.sync
            q.dma_start(
                out=out_d[st * P:(st + 1) * P, ech * CH:(ech + 1) * CH],
                in_=ob)
            yield

        def p2_items(c, interleave=True):
            its = []
            for hp in range(HP):
                for wi in (0, 1):
                    its.append(gen_p2b(c, hp, wi))
                if interleave:
                    its.append(gen_p2a(c, hp))
            if not interleave:
                for sti in range(4):
                    its.append(gen_p2a(c, sti))
            return its

        # ---- P3 ----
        fin_ready = []     # chunks whose P4 can be queued
        done_units = {c: 0 for c in range(NCH)}

        def gen_pv(php, pchk, ppts, pool=None):
            """bf16 PV chains (one MM per head x t-tile) + unit tail."""
            ntv = 4 * pchk + 4
            pool = pool if pool is not None else pvp
            tg = "pv" if pool is pvp else "prj"
            pvs = {h: pool.tile([P, CH], f32, tag=tg, name="pv")
                   for h in (0, 1)}
            n = 0
            for tt in range(ntv):
                v0 = max(0, P * (tt - 4 * pchk))
                for h in (0, 1):
                    nc.tensor.matmul(
                        pvs[h][:, v0:CH],
                        vf[:, tt, 2 * php + h, :],
                        ppts[tt // 2][h][:, tt % 2, v0:CH],
                        start=(tt == 0), stop=(tt == ntv - 1),
                        skip_group_check=True)
                    n += 1
                    if n % 4 == 0:
                        yield
            # tail: numerators -> po; per-head reciprocal denominator,
            # DRAM-bounce stride-0 broadcast (4 DMA triggers total), outT
            # scale on gpsimd
            dd = drp.tile([2, CH], f32, tag="dd", name="dd")
            po = pop.tile([P, CH], bf16, tag="po", name="po")
            bc = bcp.tile([P, CH], f32, tag="bc", name="bc")
            for h in (0, 1):
                nc.vector.tensor_copy(
                    out=po[h * DH:(h + 1) * DH, :], in_=pvs[h][0:DH, :])
                den = dnp.tile([1, CH], f32, tag="den", name="den")
                nc.vector.tensor_copy(out=den, in_=pvs[h][DH:DH + 1, :])
                rd = dnp.tile([1, CH], f32, tag="rd", name="rd")
                nc.vector.reciprocal_approx_fast(out=rd, in_=den)
                nc.sync.dma_start(out=dd[h:h + 1, :], in_=rd)
            for h in (0, 1):
                row = dd[h:h + 1, :]
                bsrc = bass.AP(
                    tensor=row.tensor, offset=row.offset,
                    ap=[[0, DH]] + list(row.ap[1:]))
                nc.sync.dma_start(
                    out=bc[h * DH:(h + 1) * DH, :], in_=bsrc)
            cs = slice(pchk * CH, (pchk + 1) * CH)
            nc.gpsimd.tensor_mul(outTs[php][:, cs], po, bc)
            done_units[pchk] += 1
            if done_units[pchk] == HP:
                fin_ready.append(pchk)

        fill_p2 = deque()
        fill_p4 = deque()
        pvgen = None

        def drain_one(q):
            while q:
                try:
                    next(q[0])
                    return 1
                except StopIteration:
                    q.popleft()
            return 0

        def drain_fill(k):
            n = 0
            for i in range(k):
                got = drain_one(fill_p2 if i % 2 == 0 else fill_p4)
                if not got:
                    got = drain_one(fill_p4 if i % 2 == 0 else fill_p2)
                n += got
                if not got:
                    break
            return n

        def emit_unit(hp, chk, pv_steps, fill_steps, selfpv=False):
            nonlocal pvgen
            nprs = 2 * chk + 2
            pts = []
            own = None
            for pr in range(nprs):
                sps = {h: scp.tile([P, 2, CH], f32, tag="sp", name="sp")
                       for h in (0, 1)}
                for j in (0, 1):
                    tt = 2 * pr + j
                    v0 = max(0, P * (tt - 4 * chk))
                    for h in (0, 1):
                        nc.tensor.matmul(
                            sps[h][:, j, v0:CH],
                            ks[:, hp, tt * P:(tt + 1) * P],
                            qp[:, 2 * hp + h,
                               chk * CH + v0:(chk + 1) * CH],
                            start=True, stop=True)
                pt = {h: ptp.tile([P, 2, CH], bf16, tag="pt", name="pt")
                      for h in (0, 1)}
                diag = pr >= 2 * chk
                for h in (0, 1):
                    if diag:
                        v00 = P * (2 * pr - 4 * chk)
                        nc.scalar.activation(
                            out=pt[h][:, 0, v00:CH], in_=sps[h][:, 0, v00:CH],
                            func=Exp, scale=SCALE)
                        nc.scalar.activation(
                            out=pt[h][:, 1, v00 + P:CH],
                            in_=sps[h][:, 1, v00 + P:CH],
                            func=Exp, scale=SCALE)
                        for j in (0, 1):
                            va = v00 + j * P
                            nc.gpsimd.tensor_mul(
                                pt[h][:, j, va:va + P],
                                pt[h][:, j, va:va + P], msk)
                    else:
                        nc.scalar.activation(
                            out=pt[h][:, :, :], in_=sps[h][:, :, :],
                            func=Exp, scale=SCALE)
                pts.append(pt)
                # filler: pending-unit PV matmuls + P2/P4 chains
                if pvgen is not None:
                    for _ in range(pv_steps):
                        try:
                            next(pvgen)
                        except StopIteration:
                            pvgen = None
                            break
                if selfpv and pvgen is None:
                    if own is None:
                        own = gen_pv(hp, chk, pts)
                    try:
                        next(own)
                    except StopIteration:
                        own = None
                drain_fill(fill_steps)
            if selfpv and own is not None:
                for _ in own:
                    pass
            return pts, (own is None and selfpv)

        # ---- main emission ----
        # head: chunk-0 P2 interleaved with chunk-0 units so the exp
        # stream starts while x/weights are still landing
        pend = None
        for hp in range(HP):
            for wi in (0, 1):
                if hp == 0:
                    emit_warm(2)
                for _ in gen_p2b(0, hp, wi):
                    pass
            if pend is not None:
                pvgen = gen_pv(pend[0], pend[1], pend[2])
            pts, _ = emit_unit(hp, 0, 3 if pend is not None else 0,
                               1 if hp > 0 else 0)
            if pvgen is not None:
                for _ in pvgen:
                    pass
                pvgen = None
            pend = (hp, 0, pts)
            if hp == 0:
                for sti in range(4):
                    for _ in gen_p2a(0, sti):
                        pass
                emit_xt(2)
                emit_wt2()
                for g in p2_items(1):
                    fill_p2.append(g)
        while drain_one(fill_p2):
            pass

        units = [(hp, chk) for chk in range(1, NCH) for hp in range(HP)]
        for hp, chk in units:
            if hp == 0 and chk + 1 < NCH:
                if chk + 2 < NCH:
                    emit_xt(chk + 2)
                for g in p2_items(chk + 1):
                    fill_p2.append(g)
            # previous unit's PV drains across this unit's prs (lag-1;
            # sp-slot pacing guarantees its exps have completed)
            nprs = 2 * chk + 2
            if pend is not None:
                pvgen = gen_pv(pend[0], pend[1], pend[2])
                pntv = 4 * pend[1] + 4
                pv_steps = (pntv // 2 + nprs - 1) // nprs + 1
            else:
                pv_steps = 0
            last = False  # zero-lag self-PV measured slightly worse
            pts, pv_done = emit_unit(hp, chk, pv_steps,
                                     2 if chk < 2 else 1, selfpv=last)
            if pvgen is not None:
                for _ in pvgen:
                    pass
                pvgen = None
            pend = None if pv_done else (hp, chk, pts)
            while fin_ready:
                c = fin_ready.pop(0)
                for ech in range(E // CH):
                    for sti in range(4):
                        fill_p4.append(gen_p4(c, ech, sti))
            # chunk boundary: next chunk's P2 must be fully emitted
            if hp == HP - 1:
                while drain_one(fill_p2):
                    pass

        # tail: last unit's PV (if not already emitted), finalize, last P4
        if pend is not None:
            for _ in gen_pv(pend[0], pend[1], pend[2], pool=prp):
                pass
        while fin_ready:
            c = fin_ready.pop(0)
            for ech in range(E // CH):
                for sti in range(4):
                    fill_p4.append(gen_p4(c, ech, sti))
        while drain_fill(64):
            pass

    nc.finalize()
    return nc


def _get_nc():
    if "nc" not in _CACHE:
        _CACHE["nc"] = _build_nc()
    return _CACHE["nc"]


def _make_in_maps(x, Wq, Wk, Wv, Wo):
    import ml_dtypes

    bf = ml_dtypes.bfloat16
    # multiplicative causal mask for a diagonal 128x128 subtile
    pcol = np.arange(P)[:, None]
    frow = np.arange(P)[None, :]
    mask = (pcol <= frow).astype(bf)
    in_maps = []
    for c in range(NCORES):
        b, half = divmod(c, 2)
        hs = slice(half * NH, (half + 1) * NH)
        in_maps.append({
            "x": np.ascontiguousarray(x[b].T.astype(bf)),
            "wq": np.ascontiguousarray(
                Wq[hs].transpose(1, 0, 2).reshape(E, NH * DH).astype(bf)),
            "wk": np.ascontiguousarray(
                Wk[hs].transpose(1, 0, 2).reshape(E, NH * DH).astype(bf)),
            "wv": np.ascontiguousarray(
                Wv[hs].transpose(1, 0, 2).reshape(E, NH * DH).astype(bf)),
            "wo": np.ascontiguousarray(
                Wo[half * NH * DH:(half + 1) * NH * DH].astype(bf)),
            "mask": mask,
        })
    return in_maps


def _ensure_ntff_hook():
    """Register the axon NTFF profile hook under antenv.axon_hooks."""
    import types
    try:
        import antenv.axon_hooks  # noqa: F401
        return
    except ImportError:
        pass
    try:
        from trn_agent_boot.trn_boot import _ntff_profile_via_ctypes
        hook = _ntff_profile_via_ctypes("/opt/axon/libaxon_pjrt.so")
    except Exception:
        hook = None
    mod = types.ModuleType("antenv.axon_hooks")
    mod.get_axon_ntff_profile_hook = lambda: hook
    mod.set_axon_ntff_profile_hook = lambda h: None
    sys.modules["antenv.axon_hooks"] = mod


def _run(inputs, trace=False):
    from concourse.bass_utils import run_bass_kernel_spmd

    if trace:
        _ensure_ntff_hook()

    x = np.asarray(inputs["x"], dtype=np.float32)
    Wq = np.asarray(inputs["Wq"], dtype=np.float32)
    Wk = np.asarray(inputs["Wk"], dtype=np.float32)
    Wv = np.asarray(inputs["Wv"], dtype=np.float32)
    Wo = np.asarray(inputs["Wo"], dtype=np.float32)
    bo = np.asarray(inputs["bo"], dtype=np.float32)

    nc = _get_nc()
    in_maps = _make_in_maps(x, Wq, Wk, Wv, Wo)
    res = run_bass_kernel_spmd(nc, in_maps, list(range(NCORES)), trace=trace)
    out = np.empty((B, S, E), dtype=np.float32)
    for b in range(B):
        out[b] = (res.results[2 * b]["out"].astype(np.float32)
                  + res.results[2 * b + 1]["out"].astype(np.float32) + bo)
    return out, res


def kernel(**inputs):
    out, _ = _run(inputs, trace=False)
    return out


# revision 34
# speedup vs baseline: 1.0138x; 1.0138x over previous
"""Trainium2 Bass kernel for causal MultiHeadAttention (B=4,S=2048,E=1024,H=16).

Sharding: 8 cores = (batch b, head-half) grid. Core c handles batch c//2 and
heads [8*(c%2), 8*(c%2)+8). Each core computes its 8 heads' attention and the
partial output projection (its 512 rows of Wo); the host sums the two fp16
partials per batch and adds the bias. ~280us/core vs the 382us v1 baseline.

Design notes (what made it fast):
  - Every matmul is a 128-partition FWL-eligible weight: scores use the
    stacked head-pair K as the stationary operand ([128 = h0 dh | h1 dh])
    against a per-head zero-padded Q moving operand (the zero half kills
    the cross-head term), so the PE runs warm 216ns/512-col MMs
    back-to-back with weight loads fully hidden. (PE row/col tiling and
    fp8 DoubleRow were measured dead ends on this hardware: non-FWL
    weight loads serialize ~100ns/MM and DoubleRow double-pumping does
    not engage, while fp8 probs cost 8x in accuracy.)
  - PV: bf16 [128,128] V weights laid out [V | ones | zeros]; one matmul
    per (head, t-tile) accumulated over the unit, partial-N on diagonal
    tiles. The ones column makes psum row 64 the softmax denominator.
  - Causality at t-tile granularity everywhere (only 62.5% of score/PV
    work is computed); the 4 diagonal 128x128 subtiles per (head, chunk)
    are fixed post-exp with one 0/1 bf16 multiply on the idle GpSimd.
  - ACT exp is the P3 pacer (~158us): emission interleaves whole P2
    projection chains and per-chunk P4 output-projection chains between
    score groups so the PE always has FWL work while ACT chews exps;
    psum pools pace scores one pr ahead of exp (PSUM: 4 banks scores,
    2 PV, 2 projections).
  - Few, large DMAs: one multi-dim descriptor per x chunk / weight pane
    (a dma_start costs ~600ns of sequencer time, so 97 startup triggers
    were ~58us of serialized pacing in earlier versions); fp16 output
    streams out per chunk; denominator reciprocal via
    reciprocal_approx_fast and a 4-trigger DRAM-bounce broadcast.
  - Warm-up matmuls during the DMA-paced first chunk keep the PE p-state
    high (cold-start MMs run 2x slow otherwise).
"""

import sys

if "/opt/trn_rl_repo" not in sys.path:
    sys.path.insert(0, "/opt/trn_rl_repo")

import numpy as np
from collections import deque
from contextlib import ExitStack

B, S, E, H = 4, 2048, 1024, 16
DH = E // H          # 64
NCORES = 8
NH = 8               # local heads per core
HP = NH // 2         # head pairs
P = 128
NE = E // P          # 8 e-tiles
NT = S // P          # 16 t-tiles
CH = 512
NCH = S // CH        # 4 q-chunks
SCALE = 1.0 / 8.0    # 1/sqrt(DH)

_CACHE = {}


def _build_nc():
    import concourse.mybir as mybir
    import concourse.tile as tile
    import concourse.bass as bass
    from concourse import bacc

    f32 = mybir.dt.float32
    f16 = mybir.dt.float16
    bf16 = mybir.dt.bfloat16
    Exp = mybir.ActivationFunctionType.Exp
    PSUM = bass.MemorySpace.PSUM

    nc = bacc.Bacc(None)
    x_d = nc.dram_tensor("x", [E, S], bf16, kind="ExternalInput")  # pre-transposed
    wq_d = nc.dram_tensor("wq", [E, NH * DH], bf16, kind="ExternalInput")
    wk_d = nc.dram_tensor("wk", [E, NH * DH], bf16, kind="ExternalInput")
    wv_d = nc.dram_tensor("wv", [E, NH * DH], bf16, kind="ExternalInput")
    wo_d = nc.dram_tensor("wo", [NH * DH, E], bf16, kind="ExternalInput")
    msk_d = nc.dram_tensor("mask", [P, P], bf16, kind="ExternalInput")
    out_d = nc.dram_tensor("out", [S, E], f16, kind="ExternalOutput")

    with ExitStack() as ctx:
        tc = ctx.enter_context(tile.TileContext(nc))
        persist = ctx.enter_context(tc.tile_pool(name="persist", bufs=1))

        qp = persist.tile([P, NH, S], bf16)            # per-head, half zero
        ks = persist.tile([P, HP, S], bf16)            # rows = stacked pair dh
        vf = persist.tile([P, NT, NH, P], bf16)        # V | ones | zeros
        msk = persist.tile([P, P], bf16)               # 0/1 causal subtile
        outTs = [persist.tile([P, S], bf16, tag=f"outT{i}", name="outT")
                 for i in range(HP)]

        # zero fills first (both engine queues are empty at t0)
        # vf ones column and zero padding (replaces v1's 4MB zz DMA)
        nc.vector.memset(vf[:, :, :, DH:DH + 1], 1.0)
        nc.vector.memset(vf[:, :, :, DH + 1:P], 0.0)
        # qp: the half of each head's 128 rows not holding Q stays zero so
        # the K=128 stacked-K score matmul drops the other head's term
        nc.gpsimd.memset(qp[DH:P, 0::2, :], 0.0)
        nc.gpsimd.memset(qp[0:DH, 1::2, :], 0.0)

        # ---- input DMAs, critical-path order ----
        # sync queue: x chunk 0 first, then the weights that feed P2
        xtp = ctx.enter_context(tc.tile_pool(name="xtp", bufs=1))
        xcs = {}

        def emit_xt(c, split=1):
            xc = xtp.tile([P, NE, CH], bf16, tag=f"xt{c % 2}", name="xt")
            step = NE // split
            for k in range(split):
                src_ap = bass.AP(
                    tensor=x_d.tensor if hasattr(x_d, 'tensor') else x_d,
                    offset=c * CH + k * step * P * S,
                    ap=[[S, P], [P * S, step], [1, CH]])
                nc.sync.dma_start(out=xc[:, k * step:(k + 1) * step, :],
                                  in_=src_ap)
            xcs[c] = xc

        emit_xt(0, split=4)
        wvall = persist.tile([P, NE, NH * DH], bf16, tag="wvall", name="wvall")
        nc.sync.dma_start(
            out=wvall,
            in_=bass.AP(
                tensor=wv_d.tensor if hasattr(wv_d, 'tensor') else wv_d,
                offset=0,
                ap=[[NH * DH, P], [P * NH * DH, NE], [1, NH * DH]]))
        emit_xt(1)
        wt2s = {}

        def emit_wt2():
            for ech in range(E // CH):
                for hp in range(HP):
                    w2 = persist.tile([P, CH], bf16, tag=f"wt2{ech}_{hp}",
                                      name="w2")
                    nc.sync.dma_start(
                        out=w2,
                        in_=wo_d[hp * P:(hp + 1) * P,
                                 ech * CH:(ech + 1) * CH])
                    wt2s[(ech, hp)] = w2

        # scalar queue (idle after startup): wq/wk per head pair + mask
        nc.scalar.dma_start(out=msk, in_=msk_d[:])
        wts = {}
        for hp in range(HP):
            for wi, wd in enumerate((wq_d, wk_d)):
                wt = persist.tile([P, NE, P], bf16, tag=f"wt{hp}{wi}",
                                  name="wt")
                nc.scalar.dma_start(
                    out=wt,
                    in_=bass.AP(
                        tensor=wd.tensor if hasattr(wd, 'tensor') else wd,
                        offset=hp * P,
                        ap=[[NH * DH, P], [P * NH * DH, NE], [1, P]]))
                wts[(hp, wi)] = wt

        # ---- pools ----
        prp = ctx.enter_context(tc.tile_pool(name="prp", bufs=2, space=PSUM))
        scp = ctx.enter_context(tc.tile_pool(name="scp", bufs=2, space=PSUM))
        pvp = ctx.enter_context(tc.tile_pool(name="pvp", bufs=2, space=PSUM))
        ptp = ctx.enter_context(tc.tile_pool(name="ptp", bufs=18))
        pop = ctx.enter_context(tc.tile_pool(name="pop", bufs=6))
        dnp = ctx.enter_context(tc.tile_pool(name="dnp", bufs=3))
        bcp = ctx.enter_context(tc.tile_pool(name="bcp", bufs=2))
        osb = ctx.enter_context(tc.tile_pool(name="osb", bufs=3))
        drp = ctx.enter_context(tc.tile_pool(name="drp", bufs=2, space="DRAM"))

        # dummy warm-up matmuls (write an unread scp column; weights = msk,
        # moving = a memset tile): keep the PE pipeline busy through the
        # DMA-paced startup so P2 chunk 0 runs at full clock
        wrm = persist.tile([P, CH], bf16, tag="wrm", name="wrm")
        nc.vector.memset(wrm, 0.0)

        def emit_warm(n):
            ws = scp.tile([P, 2, CH], f32, tag="sp", name="sp")
            for _ in range(n):
                nc.tensor.matmul(ws[:, 0, :], msk, wrm, start=True, stop=True)

        # ---- filler work items (one whole PE chain each, ~1.8us) ----
        def gen_p2b(c, hp, wi):
            ps = prp.tile([P, CH], f32, tag="prj", name="prj")
            wt = wts[(hp, wi)]
            for et in range(NE):
                nc.tensor.matmul(ps, wt[:, et, :], xcs[c][:, et, :],
                                 start=(et == 0), stop=(et == NE - 1),
                                 skip_group_check=True)
            cs = slice(c * CH, (c + 1) * CH)
            if wi == 0:
                nc.vector.tensor_copy(
                    out=qp[0:DH, 2 * hp, cs], in_=ps[0:DH, :])
                nc.vector.tensor_copy(
                    out=qp[DH:P, 2 * hp + 1, cs], in_=ps[DH:P, :])
            else:
                nc.vector.tensor_copy(out=ks[:, hp, cs], in_=ps)
            yield

        def gen_p2a(c, sti):
            st = 4 * c + sti
            ps = prp.tile([P, CH], f32, tag="prj", name="prj")
            for et in range(NE):
                nc.tensor.matmul(
                    ps, xcs[c][:, et, sti * P:(sti + 1) * P], wvall[:, et, :],
                    start=(et == 0), stop=(et == NE - 1),
                    skip_group_check=True)
            nc.vector.tensor_copy(
                out=vf[:, st, :, 0:DH],
                in_=ps.rearrange("p (h d) -> p h d", h=NH))
            yield

        def gen_p4(c, ech, sti):
            st = 4 * c + sti
            ps = prp.tile([P, CH], f32, tag="prj", name="prj")
            for hp in range(HP):
                nc.tensor.matmul(
                    ps, outTs[hp][:, st * P:(st + 1) * P], wt2s[(ech, hp)],
                    start=(hp == 0), stop=(hp == HP - 1),
                    skip_group_check=True)
            ob = osb.tile([P, CH], f16, tag="ob", name="ob")
            nc.vector.tensor_copy(out=ob, in_=ps)
            q = nc.scalar if c == NCH - 1 else nc.sync
            q.dma_start(
                out=out_d[st * P:(st + 1) * P, ech * CH:(ech + 1) * CH],
                in_=ob)
            yield

        def p2_items(c, interleave=True):
            its = []
            for hp in range(HP):
                for wi in (0, 1):
                    its.append(gen_p2b(c, hp, wi))
                if interleave:
                    its.append(gen_p2a(c, hp))
            if not interleave:
                for sti in range(4):
                    its.append(gen_p2a(c, sti))
            return its

        # ---- P3 ----
        fin_ready = []     # chunks whose P4 can be queued
        done_units = {c: 0 for c in range(NCH)}

        def gen_pv(php, pchk, ppts, pool=None):
            """bf16 PV chains (one MM per head x t-tile) + unit tail."""
            ntv = 4 * pchk + 4
            pool = pool if pool is not None else pvp
            tg = "pv" if pool is pvp else "prj"
            pvs = {h: pool.tile([P, CH], f32, tag=tg, name="pv")
                   for h in (0, 1)}
            n = 0
            for tt in range(ntv):
                v0 = max(0, P * (tt - 4 * pchk))
                for h in (0, 1):
                    nc.tensor.matmul(
                        pvs[h][:, v0:CH],
                        vf[:, tt, 2 * php + h, :],
                        ppts[tt // 2][h][:, tt % 2, v0:CH],
                        start=(tt == 0), stop=(tt == ntv - 1),
                        skip_group_check=True)
                    n += 1
                    if n % 4 == 0:
                        yield
            # tail: numerators -> po; per-head reciprocal denominator,
            # DRAM-bounce stride-0 broadcast (4 DMA triggers total), outT
            # scale on gpsimd
            dd = drp.tile([2, CH], f32, tag="dd", name="dd")
            po = pop.tile([P, CH], bf16, tag="po", name="po")
            bc = bcp.tile([P, CH], f32, tag="bc", name="bc")
            for h in (0, 1):
                nc.vector.tensor_copy(
                    out=po[h * DH:(h + 1) * DH, :], in_=pvs[h][0:DH, :])
                den = dnp.tile([1, CH], f32, tag="den", name="den")
                nc.vector.tensor_copy(out=den, in_=pvs[h][DH:DH + 1, :])
                rd = dnp.tile([1, CH], f32, tag="rd", name="rd")
                nc.vector.reciprocal_approx_fast(out=rd, in_=den)
                nc.sync.dma_start(out=dd[h:h + 1, :], in_=rd)
            for h in (0, 1):
                row = dd[h:h + 1, :]
                bsrc = bass.AP(
                    tensor=row.tensor, offset=row.offset,
                    ap=[[0, DH]] + list(row.ap[1:]))
                nc.sync.dma_start(
                    out=bc[h * DH:(h + 1) * DH, :], in_=bsrc)
            cs = slice(pchk * CH, (pchk + 1) * CH)
            nc.gpsimd.tensor_mul(outTs[php][:, cs], po, bc)
            done_units[pchk] += 1
            if done_units[pchk] == HP:
                fin_ready.append(pchk)

        fill_p2 = deque()
        fill_p4 = deque()
        pvgen = None

        def drain_one(q):
            while q:
                try:
                    next(q[0])
                    return 1
                except StopIteration:
                    q.popleft()
            return 0

        def drain_fill(k):
            n = 0
            for i in range(k):
                got = drain_one(fill_p2 if i % 2 == 0 else fill_p4)
                if not got:
                    got = drain_one(fill_p4 if i % 2 == 0 else fill_p2)
                n += got
                if not got:
                    break
            return n

        def emit_unit(hp, chk, pv_steps, fill_steps, selfpv=False):
            nonlocal pvgen
            nprs = 2 * chk + 2
            pts = []
            own = None
            for pr in range(nprs):
                sps = {h: scp.tile([P, 2, CH], f32, tag="sp", name="sp")
                       for h in (0, 1)}
                for j in (0, 1):
                    tt = 2 * pr + j
                    v0 = max(0, P * (tt - 4 * chk))
                    for h in (0, 1):
                        nc.tensor.matmul(
                            sps[h][:, j, v0:CH],
                            ks[:, hp, tt * P:(tt + 1) * P],
                            qp[:, 2 * hp + h,
                               chk * CH + v0:(chk + 1) * CH],
                            start=True, stop=True)
                pt = {h: ptp.tile([P, 2, CH], bf16, tag="pt", name="pt")
                      for h in (0, 1)}
                diag = pr >= 2 * chk
                for h in (0, 1):
                    if diag:
                        v00 = P * (2 * pr - 4 * chk)
                        nc.scalar.activation(
                            out=pt[h][:, 0, v00:CH], in_=sps[h][:, 0, v00:CH],
                            func=Exp, scale=SCALE)
                        nc.scalar.activation(
                            out=pt[h][:, 1, v00 + P:CH],
                            in_=sps[h][:, 1, v00 + P:CH],
                            func=Exp, scale=SCALE)
                        for j in (0, 1):
                            va = v00 + j * P
                            nc.gpsimd.tensor_mul(
                                pt[h][:, j, va:va + P],
                                pt[h][:, j, va:va + P], msk)
                    else:
                        nc.scalar.activation(
                            out=pt[h][:, :, :], in_=sps[h][:, :, :],
                            func=Exp, scale=SCALE)
                pts.append(pt)
                # filler: pending-unit PV matmuls + P2/P4 chains
                if pvgen is not None:
                    for _ in range(pv_steps):
                        try:
                            next(pvgen)
                        except StopIteration:
                            pvgen = None
                            break
                if selfpv and pvgen is None:
                    if own is None:
                        own = gen_pv(hp, chk, pts)
                    try:
                        next(own)
                    except StopIteration:
                        own = None
                drain_fill(fill_steps)
            if selfpv and own is not None:
                for _ in own:
                    pass
            return pts, (own is None and selfpv)

        # ---- main emission ----
        # head: P2 of chunk 0 (run to completion; first scores follow)
        for gi, g in enumerate(p2_items(0, interleave=False)):
            if gi < 4:
                emit_warm(2)
            for _ in g:
                pass

        units = [(hp, chk) for chk in range(NCH) for hp in range(HP)]
        pend = None
        for hp, chk in units:
            if hp == 0 and chk + 1 < NCH:
                if chk + 2 < NCH:
                    emit_xt(chk + 2)
                if chk == 0:
                    emit_wt2()
                for g in p2_items(chk + 1):
                    fill_p2.append(g)
            # previous unit's PV drains across this unit's prs (lag-1;
            # sp-slot pacing guarantees its exps have completed)
            nprs = 2 * chk + 2
            if pend is not None:
                pvgen = gen_pv(pend[0], pend[1], pend[2])
                pntv = 4 * pend[1] + 4
                pv_steps = (pntv // 2 + nprs - 1) // nprs + 1
            else:
                pv_steps = 0
            last = False  # zero-lag self-PV measured slightly worse
            pts, pv_done = emit_unit(hp, chk, pv_steps,
                                     2 if chk < 2 else 1, selfpv=last)
            if pvgen is not None:
                for _ in pvgen:
                    pass
                pvgen = None
            pend = None if pv_done else (hp, chk, pts)
            while fin_ready:
                c = fin_ready.pop(0)
                for ech in range(E // CH):
                    for sti in range(4):
                        fill_p4.append(gen_p4(c, ech, sti))
            # chunk boundary: next chunk's P2 must be fully emitted
            if hp == HP - 1:
                while drain_one(fill_p2):
                    pass

        # tail: last unit's PV (if not already emitted), finalize, last P4
        if pend is not None:
            for _ in gen_pv(pend[0], pend[1], pend[2], pool=prp):
                pass
        while fin_ready:
            c = fin_ready.pop(0)
            for ech in range(E // CH):
                for sti in range(4):
                    fill_p4.append(gen_p4(c, ech, sti))
        while drain_fill(64):
            pass

    nc.finalize()
    return nc


def _get_nc():
    if "nc" not in _CACHE:
        _CACHE["nc"] = _build_nc()
    return _CACHE["nc"]


def _make_in_maps(x, Wq, Wk, Wv, Wo):
    import ml_dtypes

    bf = ml_dtypes.bfloat16
    # multiplicative causal mask for a diagonal 128x128 subtile
    pcol = np.arange(P)[:, None]
    frow = np.arange(P)[None, :]
    mask = (pcol <= frow).astype(bf)
    in_maps = []
    for c in range(NCORES):
        b, half = divmod(c, 2)
        hs = slice(half * NH, (half + 1) * NH)
        in_maps.append({
            "x": np.ascontiguousarray(x[b].T.astype(bf)),
            "wq": np.ascontiguousarray(
                Wq[hs].transpose(1, 0, 2).reshape(E, NH * DH).astype(bf)),
            "wk": np.ascontiguousarray(
                Wk[hs].transpose(1, 0, 2).reshape(E, NH * DH).astype(bf)),
            "wv": np.ascontiguousarray(
                Wv[hs].transpose(1, 0, 2).reshape(E, NH * DH).astype(bf)),
            "wo": np.ascontiguousarray(
                Wo[half * NH * DH:(half + 1) * NH * DH].astype(bf)),
            "mask": mask,
        })
    return in_maps


def _ensure_ntff_hook():
    """Register the axon NTFF profile hook under antenv.axon_hooks."""
    import types
    try:
        import antenv.axon_hooks  # noqa: F401
        return
    except ImportError:
        pass
    try:
        from trn_agent_boot.trn_boot import _ntff_profile_via_ctypes
        hook = _ntff_profile_via_ctypes("/opt/axon/libaxon_pjrt.so")
    except Exception:
        hook = None
    mod = types.ModuleType("antenv.axon_hooks")
    mod.get_axon_ntff_profile_hook = lambda: hook
    mod.set_axon_ntff_profile_hook = lambda h: None
    sys.modules["antenv.axon_hooks"] = mod


def _run(inputs, trace=False):
    from concourse.bass_utils import run_bass_kernel_spmd

    if trace:
        _ensure_ntff_hook()

    x = np.asarray(inputs["x"], dtype=np.float32)
    Wq = np.asarray(inputs["Wq"], dtype=np.float32)
    Wk = np.asarray(inputs["Wk"], dtype=np.float32)
    Wv = np.asarray(inputs["Wv"], dtype=np.float32)
    Wo = np.asarray(inputs["Wo"], dtype=np.float32)
    bo = np.asarray(inputs["bo"], dtype=np.float32)

    nc = _get_nc()
    in_maps = _make_in_maps(x, Wq, Wk, Wv, Wo)
    res = run_bass_kernel_spmd(nc, in_maps, list(range(NCORES)), trace=trace)
    out = np.empty((B, S, E), dtype=np.float32)
    for b in range(B):
        out[b] = (res.results[2 * b]["out"].astype(np.float32)
                  + res.results[2 * b + 1]["out"].astype(np.float32) + bo)
    return out, res


def kernel(**inputs):
    out, _ = _run(inputs, trace=False)
    return out


# revision 35
# speedup vs baseline: 1.0301x; 1.0161x over previous
"""Trainium2 Bass kernel for causal MultiHeadAttention (B=4,S=2048,E=1024,H=16).

Sharding: 8 cores = (batch b, head-half) grid. Core c handles batch c//2 and
heads [8*(c%2), 8*(c%2)+8). Each core computes its 8 heads' attention and the
partial output projection (its 512 rows of Wo); the host sums the two fp16
partials per batch and adds the bias. ~280us/core vs the 382us v1 baseline.

Design notes (what made it fast):
  - Every matmul is a 128-partition FWL-eligible weight: scores use the
    stacked head-pair K as the stationary operand ([128 = h0 dh | h1 dh])
    against a per-head zero-padded Q moving operand (the zero half kills
    the cross-head term), so the PE runs warm 216ns/512-col MMs
    back-to-back with weight loads fully hidden. (PE row/col tiling and
    fp8 DoubleRow were measured dead ends on this hardware: non-FWL
    weight loads serialize ~100ns/MM and DoubleRow double-pumping does
    not engage, while fp8 probs cost 8x in accuracy.)
  - PV: bf16 [128,128] V weights laid out [V | ones | zeros]; one matmul
    per (head, t-tile) accumulated over the unit, partial-N on diagonal
    tiles. The ones column makes psum row 64 the softmax denominator.
  - Causality at t-tile granularity everywhere (only 62.5% of score/PV
    work is computed); the 4 diagonal 128x128 subtiles per (head, chunk)
    are fixed post-exp with one 0/1 bf16 multiply on the idle GpSimd.
  - ACT exp is the P3 pacer (~158us): emission interleaves whole P2
    projection chains and per-chunk P4 output-projection chains between
    score groups so the PE always has FWL work while ACT chews exps;
    psum pools pace scores one pr ahead of exp (PSUM: 4 banks scores,
    2 PV, 2 projections).
  - Few, large DMAs: one multi-dim descriptor per x chunk / weight pane
    (a dma_start costs ~600ns of sequencer time, so 97 startup triggers
    were ~58us of serialized pacing in earlier versions); fp16 output
    streams out per chunk; denominator reciprocal via
    reciprocal_approx_fast and a 4-trigger DRAM-bounce broadcast.
  - Warm-up matmuls during the DMA-paced first chunk keep the PE p-state
    high (cold-start MMs run 2x slow otherwise).
"""

import sys

if "/opt/trn_rl_repo" not in sys.path:
    sys.path.insert(0, "/opt/trn_rl_repo")

import numpy as np
from collections import deque
from contextlib import ExitStack

B, S, E, H = 4, 2048, 1024, 16
DH = E // H          # 64
NCORES = 8
NH = 8               # local heads per core
HP = NH // 2         # head pairs
P = 128
NE = E // P          # 8 e-tiles
NT = S // P          # 16 t-tiles
CH = 512
NCH = S // CH        # 4 q-chunks
SCALE = 1.0 / 8.0    # 1/sqrt(DH)

_CACHE = {}


def _build_nc():
    import concourse.mybir as mybir
    import concourse.tile as tile
    import concourse.bass as bass
    from concourse import bacc

    f32 = mybir.dt.float32
    f16 = mybir.dt.float16
    bf16 = mybir.dt.bfloat16
    Exp = mybir.ActivationFunctionType.Exp
    PSUM = bass.MemorySpace.PSUM

    nc = bacc.Bacc(None)
    x_d = nc.dram_tensor("x", [E, S], bf16, kind="ExternalInput")  # pre-transposed
    wq_d = nc.dram_tensor("wq", [E, NH * DH], bf16, kind="ExternalInput")
    wk_d = nc.dram_tensor("wk", [E, NH * DH], bf16, kind="ExternalInput")
    wv_d = nc.dram_tensor("wv", [E, NH * DH], bf16, kind="ExternalInput")
    wo_d = nc.dram_tensor("wo", [NH * DH, E], bf16, kind="ExternalInput")
    msk_d = nc.dram_tensor("mask", [P, P], bf16, kind="ExternalInput")
    out_d = nc.dram_tensor("out", [S, E], f16, kind="ExternalOutput")

    with ExitStack() as ctx:
        tc = ctx.enter_context(tile.TileContext(nc))
        persist = ctx.enter_context(tc.tile_pool(name="persist", bufs=1))

        qp = persist.tile([P, NH, S], bf16)            # per-head, half zero
        ks = persist.tile([P, HP, S], bf16)            # rows = stacked pair dh
        vf = persist.tile([P, NT, NH, P], bf16)        # V | ones | zeros
        msk = persist.tile([P, P], bf16)               # 0/1 causal subtile
        outTs = [persist.tile([P, S], bf16, tag=f"outT{i}", name="outT")
                 for i in range(HP)]

        # zero fills first (both engine queues are empty at t0)
        # vf ones column and zero padding (replaces v1's 4MB zz DMA)
        nc.vector.memset(vf[:, :, :, DH:DH + 1], 1.0)
        nc.vector.memset(vf[:, :, :, DH + 1:P], 0.0)
        # qp: the half of each head's 128 rows not holding Q stays zero so
        # the K=128 stacked-K score matmul drops the other head's term
        nc.gpsimd.memset(qp[DH:P, 0::2, :], 0.0)
        nc.gpsimd.memset(qp[0:DH, 1::2, :], 0.0)

        # ---- input DMAs, critical-path order ----
        # sync queue: x chunk 0 first, then the weights that feed P2
        xtp = ctx.enter_context(tc.tile_pool(name="xtp", bufs=1))
        xcs = {}

        def emit_xt(c, split=1):
            xc = xtp.tile([P, NE, CH], bf16, tag=f"xt{c % 2}", name="xt")
            step = NE // split
            for k in range(split):
                src_ap = bass.AP(
                    tensor=x_d.tensor if hasattr(x_d, 'tensor') else x_d,
                    offset=c * CH + k * step * P * S,
                    ap=[[S, P], [P * S, step], [1, CH]])
                nc.sync.dma_start(out=xc[:, k * step:(k + 1) * step, :],
                                  in_=src_ap)
            xcs[c] = xc

        emit_xt(0, split=4)
        wvall = persist.tile([P, NE, NH * DH], bf16, tag="wvall", name="wvall")
        nc.sync.dma_start(
            out=wvall,
            in_=bass.AP(
                tensor=wv_d.tensor if hasattr(wv_d, 'tensor') else wv_d,
                offset=0,
                ap=[[NH * DH, P], [P * NH * DH, NE], [1, NH * DH]]))
        emit_xt(1)
        wt2s = {}

        def emit_wt2():
            for ech in range(E // CH):
                for hp in range(HP):
                    w2 = persist.tile([P, CH], bf16, tag=f"wt2{ech}_{hp}",
                                      name="w2")
                    nc.sync.dma_start(
                        out=w2,
                        in_=wo_d[hp * P:(hp + 1) * P,
                                 ech * CH:(ech + 1) * CH])
                    wt2s[(ech, hp)] = w2

        # scalar queue (idle after startup): wq/wk per head pair + mask
        nc.scalar.dma_start(out=msk, in_=msk_d[:])
        wts = {}
        for hp in range(HP):
            for wi, wd in enumerate((wq_d, wk_d)):
                wt = persist.tile([P, NE, P], bf16, tag=f"wt{hp}{wi}",
                                  name="wt")
                nc.scalar.dma_start(
                    out=wt,
                    in_=bass.AP(
                        tensor=wd.tensor if hasattr(wd, 'tensor') else wd,
                        offset=hp * P,
                        ap=[[NH * DH, P], [P * NH * DH, NE], [1, P]]))
                wts[(hp, wi)] = wt

        # ---- pools ----
        prp = ctx.enter_context(tc.tile_pool(name="prp", bufs=2, space=PSUM))
        scp = ctx.enter_context(tc.tile_pool(name="scp", bufs=2, space=PSUM))
        pvp = ctx.enter_context(tc.tile_pool(name="pvp", bufs=2, space=PSUM))
        ptp = ctx.enter_context(tc.tile_pool(name="ptp", bufs=18))
        pop = ctx.enter_context(tc.tile_pool(name="pop", bufs=6))
        dnp = ctx.enter_context(tc.tile_pool(name="dnp", bufs=3))
        bcp = ctx.enter_context(tc.tile_pool(name="bcp", bufs=2))
        osb = ctx.enter_context(tc.tile_pool(name="osb", bufs=3))
        drp = ctx.enter_context(tc.tile_pool(name="drp", bufs=2, space="DRAM"))

        # dummy warm-up matmuls (write an unread scp column; weights = msk,
        # moving = a memset tile): keep the PE pipeline busy through the
        # DMA-paced startup so P2 chunk 0 runs at full clock
        wrm = persist.tile([P, CH], bf16, tag="wrm", name="wrm")
        nc.vector.memset(wrm, 0.0)

        def emit_warm(n):
            ws = scp.tile([P, 2, CH], f32, tag="sp", name="sp")
            for _ in range(n):
                nc.tensor.matmul(ws[:, 0, :], msk, wrm, start=True, stop=True)

        # ---- filler work items (one whole PE chain each, ~1.8us) ----
        def gen_p2b(c, hp, wi):
            ps = prp.tile([P, CH], f32, tag="prj", name="prj")
            wt = wts[(hp, wi)]
            for et in range(NE):
                nc.tensor.matmul(ps, wt[:, et, :], xcs[c][:, et, :],
                                 start=(et == 0), stop=(et == NE - 1),
                                 skip_group_check=True)
            cs = slice(c * CH, (c + 1) * CH)
            if wi == 0:
                nc.vector.tensor_copy(
                    out=qp[0:DH, 2 * hp, cs], in_=ps[0:DH, :])
                nc.vector.tensor_copy(
                    out=qp[DH:P, 2 * hp + 1, cs], in_=ps[DH:P, :])
            else:
                nc.vector.tensor_copy(out=ks[:, hp, cs], in_=ps)
            yield

        def gen_p2a(c, sti):
            st = 4 * c + sti
            ps = prp.tile([P, CH], f32, tag="prj", name="prj")
            for et in range(NE):
                nc.tensor.matmul(
                    ps, xcs[c][:, et, sti * P:(sti + 1) * P], wvall[:, et, :],
                    start=(et == 0), stop=(et == NE - 1),
                    skip_group_check=True)
            nc.vector.tensor_copy(
                out=vf[:, st, :, 0:DH],
                in_=ps.rearrange("p (h d) -> p h d", h=NH))
            yield

        def gen_p4(c, ech, sti):
            st = 4 * c + sti
            ps = prp.tile([P, CH], f32, tag="prj", name="prj")
            for hp in range(HP):
                nc.tensor.matmul(
                    ps, outTs[hp][:, st * P:(st + 1) * P], wt2s[(ech, hp)],
                    start=(hp == 0), stop=(hp == HP - 1),
                    skip_group_check=True)
            ob = osb.tile([P, CH], f16, tag="ob", name="ob")
            nc.vector.tensor_copy(out=ob, in_=ps)
            q = nc.scalar if c == NCH - 1 else nc.sync
            q.dma_start(
                out=out_d[st * P:(st + 1) * P, ech * CH:(ech + 1) * CH],
                in_=ob)
            yield

        def p2_items(c, interleave=True):
            its = []
            for hp in range(HP):
                for wi in (0, 1):
                    its.append(gen_p2b(c, hp, wi))
                if interleave:
                    its.append(gen_p2a(c, hp))
            if not interleave:
                for sti in range(4):
                    its.append(gen_p2a(c, sti))
            return its

        # ---- P3 ----
        fin_ready = []     # chunks whose P4 can be queued
        done_units = {c: 0 for c in range(NCH)}

        def gen_pv(php, pchk, ppts, pool=None):
            """bf16 PV chains (one MM per head x t-tile) + unit tail."""
            ntv = 4 * pchk + 4
            pool = pool if pool is not None else pvp
            tg = "pv" if pool is pvp else "prj"
            pvs = {h: pool.tile([P, CH], f32, tag=tg, name="pv")
                   for h in (0, 1)}
            n = 0
            for tt in range(ntv):
                v0 = max(0, P * (tt - 4 * pchk))
                for h in (0, 1):
                    nc.tensor.matmul(
                        pvs[h][:, v0:CH],
                        vf[:, tt, 2 * php + h, :],
                        ppts[tt // 2][h][:, tt % 2, v0:CH],
                        start=(tt == 0), stop=(tt == ntv - 1),
                        skip_group_check=True)
                    n += 1
                    if n % 4 == 0:
                        yield
            # tail: numerators -> po; per-head reciprocal denominator,
            # DRAM-bounce stride-0 broadcast (4 DMA triggers total), outT
            # scale on gpsimd
            dd = drp.tile([2, CH], f32, tag="dd", name="dd")
            po = pop.tile([P, CH], bf16, tag="po", name="po")
            bc = bcp.tile([P, CH], f32, tag="bc", name="bc")
            for h in (0, 1):
                nc.vector.tensor_copy(
                    out=po[h * DH:(h + 1) * DH, :], in_=pvs[h][0:DH, :])
                den = dnp.tile([1, CH], f32, tag="den", name="den")
                nc.vector.tensor_copy(out=den, in_=pvs[h][DH:DH + 1, :])
                rd = dnp.tile([1, CH], f32, tag="rd", name="rd")
                nc.vector.reciprocal_approx_fast(out=rd, in_=den)
                nc.sync.dma_start(out=dd[h:h + 1, :], in_=rd)
            for h in (0, 1):
                row = dd[h:h + 1, :]
                bsrc = bass.AP(
                    tensor=row.tensor, offset=row.offset,
                    ap=[[0, DH]] + list(row.ap[1:]))
                nc.sync.dma_start(
                    out=bc[h * DH:(h + 1) * DH, :], in_=bsrc)
            cs = slice(pchk * CH, (pchk + 1) * CH)
            mule = nc.vector if pchk == NCH - 1 else nc.gpsimd
            mule.tensor_mul(outTs[php][:, cs], po, bc)
            done_units[pchk] += 1
            if done_units[pchk] == HP:
                fin_ready.append(pchk)

        fill_p2 = deque()
        fill_p4 = deque()
        pvgen = None

        def drain_one(q):
            while q:
                try:
                    next(q[0])
                    return 1
                except StopIteration:
                    q.popleft()
            return 0

        def drain_fill(k):
            n = 0
            for i in range(k):
                got = drain_one(fill_p2 if i % 2 == 0 else fill_p4)
                if not got:
                    got = drain_one(fill_p4 if i % 2 == 0 else fill_p2)
                n += got
                if not got:
                    break
            return n

        def emit_unit(hp, chk, pv_steps, fill_steps, selfpv=False):
            nonlocal pvgen
            nprs = 2 * chk + 2
            pts = []
            own = None
            for pr in range(nprs):
                sps = {h: scp.tile([P, 2, CH], f32, tag="sp", name="sp")
                       for h in (0, 1)}
                for j in (0, 1):
                    tt = 2 * pr + j
                    v0 = max(0, P * (tt - 4 * chk))
                    for h in (0, 1):
                        nc.tensor.matmul(
                            sps[h][:, j, v0:CH],
                            ks[:, hp, tt * P:(tt + 1) * P],
                            qp[:, 2 * hp + h,
                               chk * CH + v0:(chk + 1) * CH],
                            start=True, stop=True)
                pt = {h: ptp.tile([P, 2, CH], bf16, tag="pt", name="pt")
                      for h in (0, 1)}
                diag = pr >= 2 * chk
                for h in (0, 1):
                    if diag:
                        v00 = P * (2 * pr - 4 * chk)
                        nc.scalar.activation(
                            out=pt[h][:, :, v00:CH], in_=sps[h][:, :, v00:CH],
                            func=Exp, scale=SCALE)
                        for j in (0, 1):
                            va = v00 + j * P
                            nc.gpsimd.tensor_mul(
                                pt[h][:, j, va:va + P],
                                pt[h][:, j, va:va + P], msk)
                    else:
                        nc.scalar.activation(
                            out=pt[h][:, :, :], in_=sps[h][:, :, :],
                            func=Exp, scale=SCALE)
                pts.append(pt)
                # filler: pending-unit PV matmuls + P2/P4 chains
                if pvgen is not None:
                    for _ in range(pv_steps):
                        try:
                            next(pvgen)
                        except StopIteration:
                            pvgen = None
                            break
                if selfpv and pvgen is None:
                    if own is None:
                        own = gen_pv(hp, chk, pts)
                    try:
                        next(own)
                    except StopIteration:
                        own = None
                drain_fill(fill_steps)
            if selfpv and own is not None:
                for _ in own:
                    pass
            return pts, (own is None and selfpv)

        # ---- main emission ----
        # head: P2 of chunk 0 (run to completion; first scores follow)
        for gi, g in enumerate(p2_items(0, interleave=False)):
            if gi < 4:
                emit_warm(2)
            for _ in g:
                pass

        units = [(hp, chk) for chk in range(NCH) for hp in range(HP)]
        pend = None
        for hp, chk in units:
            if hp == 0 and chk + 1 < NCH:
                if chk + 2 < NCH:
                    emit_xt(chk + 2)
                if chk == 0:
                    emit_wt2()
                for g in p2_items(chk + 1):
                    fill_p2.append(g)
            # previous unit's PV drains across this unit's prs (lag-1;
            # sp-slot pacing guarantees its exps have completed)
            nprs = 2 * chk + 2
            if pend is not None:
                pvgen = gen_pv(pend[0], pend[1], pend[2])
                pntv = 4 * pend[1] + 4
                pv_steps = (pntv // 2 + nprs - 1) // nprs + 1
            else:
                pv_steps = 0
            last = False  # zero-lag self-PV measured slightly worse
            pts, pv_done = emit_unit(hp, chk, pv_steps,
                                     2 if chk < 2 else 1, selfpv=last)
            if pvgen is not None:
                for _ in pvgen:
                    pass
                pvgen = None
            pend = None if pv_done else (hp, chk, pts)
            while fin_ready:
                c = fin_ready.pop(0)
                for ech in range(E // CH):
                    for sti in range(4):
                        fill_p4.append(gen_p4(c, ech, sti))
            # chunk boundary: next chunk's P2 must be fully emitted
            if hp == HP - 1:
                while drain_one(fill_p2):
                    pass

        # tail: last unit's PV (if not already emitted), finalize, last P4
        if pend is not None:
            for _ in gen_pv(pend[0], pend[1], pend[2], pool=prp):
                pass
        while fin_ready:
            c = fin_ready.pop(0)
            for ech in range(E // CH):
                for sti in range(4):
                    fill_p4.append(gen_p4(c, ech, sti))
        while drain_fill(64):
            pass

    nc.finalize()
    return nc


def _get_nc():
    if "nc" not in _CACHE:
        _CACHE["nc"] = _build_nc()
    return _CACHE["nc"]


def _make_in_maps(x, Wq, Wk, Wv, Wo):
    import ml_dtypes

    bf = ml_dtypes.bfloat16
    # multiplicative causal mask for a diagonal 128x128 subtile
    pcol = np.arange(P)[:, None]
    frow = np.arange(P)[None, :]
    mask = (pcol <= frow).astype(bf)
    in_maps = []
    for c in range(NCORES):
        b, half = divmod(c, 2)
        hs = slice(half * NH, (half + 1) * NH)
        in_maps.append({
            "x": np.ascontiguousarray(x[b].T.astype(bf)),
            "wq": np.ascontiguousarray(
                Wq[hs].transpose(1, 0, 2).reshape(E, NH * DH).astype(bf)),
            "wk": np.ascontiguousarray(
                Wk[hs].transpose(1, 0, 2).reshape(E, NH * DH).astype(bf)),
            "wv": np.ascontiguousarray(
                Wv[hs].transpose(1, 0, 2).reshape(E, NH * DH).astype(bf)),
            "wo": np.ascontiguousarray(
                Wo[half * NH * DH:(half + 1) * NH * DH].astype(bf)),
            "mask": mask,
        })
    return in_maps


def _ensure_ntff_hook():
    """Register the axon NTFF profile hook under antenv.axon_hooks."""
    import types
    try:
        import antenv.axon_hooks  # noqa: F401
        return
    except ImportError:
        pass
    try:
        from trn_agent_boot.trn_boot import _ntff_profile_via_ctypes
        hook = _ntff_profile_via_ctypes("/opt/axon/libaxon_pjrt.so")
    except Exception:
        hook = None
    mod = types.ModuleType("antenv.axon_hooks")
    mod.get_axon_ntff_profile_hook = lambda: hook
    mod.set_axon_ntff_profile_hook = lambda h: None
    sys.modules["antenv.axon_hooks"] = mod


def _run(inputs, trace=False):
    from concourse.bass_utils import run_bass_kernel_spmd

    if trace:
        _ensure_ntff_hook()

    x = np.asarray(inputs["x"], dtype=np.float32)
    Wq = np.asarray(inputs["Wq"], dtype=np.float32)
    Wk = np.asarray(inputs["Wk"], dtype=np.float32)
    Wv = np.asarray(inputs["Wv"], dtype=np.float32)
    Wo = np.asarray(inputs["Wo"], dtype=np.float32)
    bo = np.asarray(inputs["bo"], dtype=np.float32)

    nc = _get_nc()
    in_maps = _make_in_maps(x, Wq, Wk, Wv, Wo)
    res = run_bass_kernel_spmd(nc, in_maps, list(range(NCORES)), trace=trace)
    out = np.empty((B, S, E), dtype=np.float32)
    for b in range(B):
        out[b] = (res.results[2 * b]["out"].astype(np.float32)
                  + res.results[2 * b + 1]["out"].astype(np.float32) + bo)
    return out, res


def kernel(**inputs):
    out, _ = _run(inputs, trace=False)
    return out


# revision 38
# speedup vs baseline: 1.0439x; 1.0134x over previous
"""Trainium2 Bass kernel for causal MultiHeadAttention (B=4,S=2048,E=1024,H=16).

Sharding: 8 cores = (batch b, head-half) grid. Core c handles batch c//2 and
heads [8*(c%2), 8*(c%2)+8). Each core computes its 8 heads' attention and the
partial output projection (its 512 rows of Wo); the host sums the two fp16
partials per batch and adds the bias. ~280us/core vs the 382us v1 baseline.

Design notes (what made it fast):
  - Every matmul is a 128-partition FWL-eligible weight: scores use the
    stacked head-pair K as the stationary operand ([128 = h0 dh | h1 dh])
    against a per-head zero-padded Q moving operand (the zero half kills
    the cross-head term), so the PE runs warm 216ns/512-col MMs
    back-to-back with weight loads fully hidden. (PE row/col tiling and
    fp8 DoubleRow were measured dead ends on this hardware: non-FWL
    weight loads serialize ~100ns/MM and DoubleRow double-pumping does
    not engage, while fp8 probs cost 8x in accuracy.)
  - PV: bf16 [128,128] V weights laid out [V | ones | zeros]; one matmul
    per (head, t-tile) accumulated over the unit, partial-N on diagonal
    tiles. The ones column makes psum row 64 the softmax denominator.
  - Causality at t-tile granularity everywhere (only 62.5% of score/PV
    work is computed); the 4 diagonal 128x128 subtiles per (head, chunk)
    are fixed post-exp with one 0/1 bf16 multiply on the idle GpSimd.
  - ACT exp is the P3 pacer (~158us): emission interleaves whole P2
    projection chains and per-chunk P4 output-projection chains between
    score groups so the PE always has FWL work while ACT chews exps;
    psum pools pace scores one pr ahead of exp (PSUM: 4 banks scores,
    2 PV, 2 projections).
  - Few, large DMAs: one multi-dim descriptor per x chunk / weight pane
    (a dma_start costs ~600ns of sequencer time, so 97 startup triggers
    were ~58us of serialized pacing in earlier versions); fp16 output
    streams out per chunk; denominator reciprocal via
    reciprocal_approx_fast and a 4-trigger DRAM-bounce broadcast.
  - Warm-up matmuls during the DMA-paced first chunk keep the PE p-state
    high (cold-start MMs run 2x slow otherwise).
"""

import sys

if "/opt/trn_rl_repo" not in sys.path:
    sys.path.insert(0, "/opt/trn_rl_repo")

import numpy as np
from collections import deque
from contextlib import ExitStack

B, S, E, H = 4, 2048, 1024, 16
DH = E // H          # 64
NCORES = 8
NH = 8               # local heads per core
HP = NH // 2         # head pairs
P = 128
NE = E // P          # 8 e-tiles
NT = S // P          # 16 t-tiles
CH = 512
NCH = S // CH        # 4 q-chunks
SCALE = 1.0 / 8.0    # 1/sqrt(DH)

_CACHE = {}


def _build_nc():
    import concourse.mybir as mybir
    import concourse.tile as tile
    import concourse.bass as bass
    from concourse import bacc

    f32 = mybir.dt.float32
    f16 = mybir.dt.float16
    bf16 = mybir.dt.bfloat16
    Exp = mybir.ActivationFunctionType.Exp
    PSUM = bass.MemorySpace.PSUM

    nc = bacc.Bacc(None)
    x_d = nc.dram_tensor("x", [E, S], bf16, kind="ExternalInput")  # pre-transposed
    wq_d = nc.dram_tensor("wq", [E, NH * DH], bf16, kind="ExternalInput")
    wk_d = nc.dram_tensor("wk", [E, NH * DH], bf16, kind="ExternalInput")
    wv_d = nc.dram_tensor("wv", [E, NH * DH], bf16, kind="ExternalInput")
    wo_d = nc.dram_tensor("wo", [NH * DH, E], bf16, kind="ExternalInput")
    msk_d = nc.dram_tensor("mask", [P, P], bf16, kind="ExternalInput")
    out_d = nc.dram_tensor("out", [S, E], f16, kind="ExternalOutput")

    with ExitStack() as ctx:
        tc = ctx.enter_context(tile.TileContext(nc))
        persist = ctx.enter_context(tc.tile_pool(name="persist", bufs=1))

        qp = persist.tile([P, NH, S], bf16)            # per-head, half zero
        ks = persist.tile([P, HP, S], bf16)            # rows = stacked pair dh
        vf = persist.tile([P, NT, NH, P], bf16)        # V | ones | zeros
        msk = persist.tile([P, P], bf16)               # 0/1 causal subtile
        outTs = [persist.tile([P, S], bf16, tag=f"outT{i}", name="outT")
                 for i in range(HP)]

        # zero fills first (both engine queues are empty at t0)
        # vf ones column and zero padding (replaces v1's 4MB zz DMA)
        nc.vector.memset(vf[:, :, :, DH:DH + 1], 1.0)
        nc.vector.memset(vf[:, :, :, DH + 1:P], 0.0)
        # qp: the half of each head's 128 rows not holding Q stays zero so
        # the K=128 stacked-K score matmul drops the other head's term
        nc.gpsimd.memset(qp[DH:P, 0::2, :], 0.0)
        nc.gpsimd.memset(qp[0:DH, 1::2, :], 0.0)

        # ---- input DMAs, critical-path order ----
        # sync queue: x chunk 0 first, then the weights that feed P2
        xtp = ctx.enter_context(tc.tile_pool(name="xtp", bufs=1))
        xcs = {}

        def emit_xt(c, split=1):
            xc = xtp.tile([P, NE, CH], bf16, tag=f"xt{c % 2}", name="xt")
            step = NE // split
            for k in range(split):
                src_ap = bass.AP(
                    tensor=x_d.tensor if hasattr(x_d, 'tensor') else x_d,
                    offset=c * CH + k * step * P * S,
                    ap=[[S, P], [P * S, step], [1, CH]])
                nc.sync.dma_start(out=xc[:, k * step:(k + 1) * step, :],
                                  in_=src_ap)
            xcs[c] = xc

        emit_xt(0, split=4)
        wvall = persist.tile([P, NE, NH * DH], bf16, tag="wvall", name="wvall")
        nc.sync.dma_start(
            out=wvall,
            in_=bass.AP(
                tensor=wv_d.tensor if hasattr(wv_d, 'tensor') else wv_d,
                offset=0,
                ap=[[NH * DH, P], [P * NH * DH, NE], [1, NH * DH]]))
        emit_xt(1)
        wt2s = {}

        def emit_wt2():
            for ech in range(E // CH):
                for hp in range(HP):
                    w2 = persist.tile([P, CH], bf16, tag=f"wt2{ech}_{hp}",
                                      name="w2")
                    nc.sync.dma_start(
                        out=w2,
                        in_=wo_d[hp * P:(hp + 1) * P,
                                 ech * CH:(ech + 1) * CH])
                    wt2s[(ech, hp)] = w2

        # scalar queue (idle after startup): wq/wk per head pair + mask
        nc.scalar.dma_start(out=msk, in_=msk_d[:])
        wts = {}
        for hp in range(HP):
            for wi, wd in enumerate((wq_d, wk_d)):
                wt = persist.tile([P, NE, P], bf16, tag=f"wt{hp}{wi}",
                                  name="wt")
                nc.scalar.dma_start(
                    out=wt,
                    in_=bass.AP(
                        tensor=wd.tensor if hasattr(wd, 'tensor') else wd,
                        offset=hp * P,
                        ap=[[NH * DH, P], [P * NH * DH, NE], [1, P]]))
                wts[(hp, wi)] = wt

        # ---- pools ----
        prp = ctx.enter_context(tc.tile_pool(name="prp", bufs=2, space=PSUM))
        scp = ctx.enter_context(tc.tile_pool(name="scp", bufs=2, space=PSUM))
        pvp = ctx.enter_context(tc.tile_pool(name="pvp", bufs=2, space=PSUM))
        ptp = ctx.enter_context(tc.tile_pool(name="ptp", bufs=17))
        pop = ctx.enter_context(tc.tile_pool(name="pop", bufs=6))
        dnp = ctx.enter_context(tc.tile_pool(name="dnp", bufs=3))
        bcp = ctx.enter_context(tc.tile_pool(name="bcp", bufs=2))
        osb = ctx.enter_context(tc.tile_pool(name="osb", bufs=3))
        drp = ctx.enter_context(tc.tile_pool(name="drp", bufs=2, space="DRAM"))

        # dummy warm-up matmuls (write an unread scp column; weights = msk,
        # moving = a memset tile): keep the PE pipeline busy through the
        # DMA-paced startup so P2 chunk 0 runs at full clock
        wrm = persist.tile([P, CH], bf16, tag="wrm", name="wrm")
        nc.vector.memset(wrm, 0.0)
        lbc = persist.tile([1, DH], f32, tag="lbc", name="lbc")
        nc.vector.memset(lbc, 1.0)

        def emit_warm(n):
            ws = scp.tile([P, 2, CH], f32, tag="sp", name="sp")
            for _ in range(n):
                nc.tensor.matmul(ws[:, 0, :], msk, wrm, start=True, stop=True)

        # ---- filler work items (one whole PE chain each, ~1.8us) ----
        def gen_p2b(c, hp, wi):
            ps = prp.tile([P, CH], f32, tag="prj", name="prj")
            wt = wts[(hp, wi)]
            for et in range(NE):
                nc.tensor.matmul(ps, wt[:, et, :], xcs[c][:, et, :],
                                 start=(et == 0), stop=(et == NE - 1),
                                 skip_group_check=True)
            cs = slice(c * CH, (c + 1) * CH)
            if wi == 0:
                nc.vector.tensor_copy(
                    out=qp[0:DH, 2 * hp, cs], in_=ps[0:DH, :])
                nc.vector.tensor_copy(
                    out=qp[DH:P, 2 * hp + 1, cs], in_=ps[DH:P, :])
            else:
                nc.vector.tensor_copy(out=ks[:, hp, cs], in_=ps)
            yield

        def gen_p2a(c, sti):
            st = 4 * c + sti
            ps = prp.tile([P, CH], f32, tag="prj", name="prj")
            for et in range(NE):
                nc.tensor.matmul(
                    ps, xcs[c][:, et, sti * P:(sti + 1) * P], wvall[:, et, :],
                    start=(et == 0), stop=(et == NE - 1),
                    skip_group_check=True)
            nc.vector.tensor_copy(
                out=vf[:, st, :, 0:DH],
                in_=ps.rearrange("p (h d) -> p h d", h=NH))
            yield

        def gen_p4(c, ech, sti):
            st = 4 * c + sti
            ps = prp.tile([P, CH], f32, tag="prj", name="prj")
            for hp in range(HP):
                nc.tensor.matmul(
                    ps, outTs[hp][:, st * P:(st + 1) * P], wt2s[(ech, hp)],
                    start=(hp == 0), stop=(hp == HP - 1),
                    skip_group_check=True)
            ob = osb.tile([P, CH], f16, tag="ob", name="ob")
            nc.vector.tensor_copy(out=ob, in_=ps)
            q = nc.scalar if c == NCH - 1 else nc.sync
            q.dma_start(
                out=out_d[st * P:(st + 1) * P, ech * CH:(ech + 1) * CH],
                in_=ob)
            yield

        def p2_items(c, interleave=True):
            its = []
            for hp in range(HP):
                for wi in (0, 1):
                    its.append(gen_p2b(c, hp, wi))
                if interleave:
                    its.append(gen_p2a(c, hp))
            if not interleave:
                for sti in range(4):
                    its.append(gen_p2a(c, sti))
            return its

        # ---- P3 ----
        fin_ready = []     # chunks whose P4 can be queued
        done_units = {c: 0 for c in range(NCH)}

        def gen_pv(php, pchk, ppts, pool=None):
            """bf16 PV chains (one MM per head x t-tile) + unit tail."""
            ntv = 4 * pchk + 4
            pool = pool if pool is not None else pvp
            tg = "pv" if pool is pvp else "prj"
            pvs = {h: pool.tile([P, CH], f32, tag=tg, name="pv")
                   for h in (0, 1)}
            n = 0
            for tt in range(ntv):
                v0 = max(0, P * (tt - 4 * pchk))
                for h in (0, 1):
                    nc.tensor.matmul(
                        pvs[h][:, v0:CH],
                        vf[:, tt, 2 * php + h, :],
                        ppts[tt // 2][h][:, tt % 2, v0:CH],
                        start=(tt == 0), stop=(tt == ntv - 1),
                        skip_group_check=True)
                    n += 1
                    if n % 4 == 0:
                        yield
            # tail: numerators -> po; per-head reciprocal denominator,
            # DRAM-bounce stride-0 broadcast (4 DMA triggers total), outT
            # scale on gpsimd
            po = pop.tile([P, CH], bf16, tag="po", name="po")
            cs = slice(pchk * CH, (pchk + 1) * CH)
            final = php == HP - 1 and pchk == NCH - 1
            if final:
                # tail path: broadcast 1/den via a tiny fp32 outer-product
                # matmul on the (idle) PE instead of the DRAM bounce
                bcps = prp.tile([P, CH], f32, tag="prj", name="prj")
                for h in (0, 1):
                    nc.vector.tensor_copy(
                        out=po[h * DH:(h + 1) * DH, :], in_=pvs[h][0:DH, :])
                    den = dnp.tile([1, CH], f32, tag="den", name="den")
                    nc.vector.tensor_copy(out=den, in_=pvs[h][DH:DH + 1, :])
                    rd = dnp.tile([1, CH], f32, tag="rd", name="rd")
                    nc.vector.reciprocal_approx_fast(out=rd, in_=den)
                    nc.tensor.matmul(
                        bcps[h * DH:(h + 1) * DH, :], lbc, rd,
                        start=True, stop=True, skip_group_check=True)
                nc.vector.tensor_mul(outTs[php][:, cs], po, bcps)
            else:
                dd = drp.tile([2, CH], f32, tag="dd", name="dd")
                bc = bcp.tile([P, CH], f32, tag="bc", name="bc")
                for h in (0, 1):
                    nc.vector.tensor_copy(
                        out=po[h * DH:(h + 1) * DH, :], in_=pvs[h][0:DH, :])
                    den = dnp.tile([1, CH], f32, tag="den", name="den")
                    nc.vector.tensor_copy(out=den, in_=pvs[h][DH:DH + 1, :])
                    rd = dnp.tile([1, CH], f32, tag="rd", name="rd")
                    nc.vector.reciprocal_approx_fast(out=rd, in_=den)
                    nc.sync.dma_start(out=dd[h:h + 1, :], in_=rd)
                for h in (0, 1):
                    row = dd[h:h + 1, :]
                    bsrc = bass.AP(
                        tensor=row.tensor, offset=row.offset,
                        ap=[[0, DH]] + list(row.ap[1:]))
                    nc.sync.dma_start(
                        out=bc[h * DH:(h + 1) * DH, :], in_=bsrc)
                nc.gpsimd.tensor_mul(outTs[php][:, cs], po, bc)
            done_units[pchk] += 1
            if done_units[pchk] == HP:
                fin_ready.append(pchk)

        fill_p2 = deque()
        fill_p4 = deque()
        pvgen = None

        def drain_one(q):
            while q:
                try:
                    next(q[0])
                    return 1
                except StopIteration:
                    q.popleft()
            return 0

        def drain_fill(k):
            n = 0
            for i in range(k):
                got = drain_one(fill_p2 if i % 2 == 0 else fill_p4)
                if not got:
                    got = drain_one(fill_p4 if i % 2 == 0 else fill_p2)
                n += got
                if not got:
                    break
            return n

        def emit_unit(hp, chk, pv_steps, fill_steps, selfpv=False):
            nonlocal pvgen
            nprs = 2 * chk + 2
            pts = []
            own = None
            for pr in range(nprs):
                sps = {h: scp.tile([P, 2, CH], f32, tag="sp", name="sp")
                       for h in (0, 1)}
                for j in (0, 1):
                    tt = 2 * pr + j
                    v0 = max(0, P * (tt - 4 * chk))
                    for h in (0, 1):
                        nc.tensor.matmul(
                            sps[h][:, j, v0:CH],
                            ks[:, hp, tt * P:(tt + 1) * P],
                            qp[:, 2 * hp + h,
                               chk * CH + v0:(chk + 1) * CH],
                            start=True, stop=True)
                pt = {h: ptp.tile([P, 2, CH], bf16, tag="pt", name="pt")
                      for h in (0, 1)}
                diag = pr >= 2 * chk
                for h in (0, 1):
                    if diag:
                        v00 = P * (2 * pr - 4 * chk)
                        nc.scalar.activation(
                            out=pt[h][:, :, v00:CH], in_=sps[h][:, :, v00:CH],
                            func=Exp, scale=SCALE)
                        for j in (0, 1):
                            va = v00 + j * P
                            nc.gpsimd.tensor_mul(
                                pt[h][:, j, va:va + P],
                                pt[h][:, j, va:va + P], msk)
                    else:
                        nc.scalar.activation(
                            out=pt[h][:, :, :], in_=sps[h][:, :, :],
                            func=Exp, scale=SCALE)
                pts.append(pt)
                # filler: pending-unit PV matmuls + P2/P4 chains
                if pvgen is not None:
                    for _ in range(pv_steps):
                        try:
                            next(pvgen)
                        except StopIteration:
                            pvgen = None
                            break
                if selfpv and pvgen is None:
                    if own is None:
                        own = gen_pv(hp, chk, pts)
                    try:
                        next(own)
                    except StopIteration:
                        own = None
                drain_fill(fill_steps)
            if selfpv and own is not None:
                for _ in own:
                    pass
            return pts, (own is None and selfpv)

        # ---- main emission ----
        # head: P2 of chunk 0 (run to completion; first scores follow)
        for gi, g in enumerate(p2_items(0, interleave=False)):
            if gi < 4:
                emit_warm(2)
            for _ in g:
                pass

        units = [(hp, chk) for chk in range(NCH) for hp in range(HP)]
        pend = None
        for hp, chk in units:
            if hp == 0 and chk + 1 < NCH:
                if chk + 2 < NCH:
                    emit_xt(chk + 2)
                if chk == 0:
                    emit_wt2()
                for g in p2_items(chk + 1):
                    fill_p2.append(g)
            # previous unit's PV drains across this unit's prs (lag-1;
            # sp-slot pacing guarantees its exps have completed)
            nprs = 2 * chk + 2
            if pend is not None:
                pvgen = gen_pv(pend[0], pend[1], pend[2])
                pntv = 4 * pend[1] + 4
                pv_steps = (pntv // 2 + nprs - 1) // nprs + 1
            else:
                pv_steps = 0
            last = False  # zero-lag self-PV measured slightly worse
            pts, pv_done = emit_unit(hp, chk, pv_steps,
                                     2 if chk < 2 else 1, selfpv=last)
            if pvgen is not None:
                for _ in pvgen:
                    pass
                pvgen = None
            pend = None if pv_done else (hp, chk, pts)
            while fin_ready:
                c = fin_ready.pop(0)
                for ech in range(E // CH):
                    for sti in range(4):
                        fill_p4.append(gen_p4(c, ech, sti))
            # chunk boundary: next chunk's P2 must be fully emitted
            if hp == HP - 1:
                while drain_one(fill_p2):
                    pass

        # tail: last unit's PV (if not already emitted), finalize, last P4
        if pend is not None:
            for _ in gen_pv(pend[0], pend[1], pend[2], pool=prp):
                pass
        while fin_ready:
            c = fin_ready.pop(0)
            for ech in range(E // CH):
                for sti in range(4):
                    fill_p4.append(gen_p4(c, ech, sti))
        while drain_fill(64):
            pass

    nc.finalize()
    return nc


def _get_nc():
    if "nc" not in _CACHE:
        _CACHE["nc"] = _build_nc()
    return _CACHE["nc"]


def _make_in_maps(x, Wq, Wk, Wv, Wo):
    import ml_dtypes

    bf = ml_dtypes.bfloat16
    # multiplicative causal mask for a diagonal 128x128 subtile
    pcol = np.arange(P)[:, None]
    frow = np.arange(P)[None, :]
    mask = (pcol <= frow).astype(bf)
    in_maps = []
    for c in range(NCORES):
        b, half = divmod(c, 2)
        hs = slice(half * NH, (half + 1) * NH)
        in_maps.append({
            "x": np.ascontiguousarray(x[b].T.astype(bf)),
            "wq": np.ascontiguousarray(
                Wq[hs].transpose(1, 0, 2).reshape(E, NH * DH).astype(bf)),
            "wk": np.ascontiguousarray(
                Wk[hs].transpose(1, 0, 2).reshape(E, NH * DH).astype(bf)),
            "wv": np.ascontiguousarray(
                Wv[hs].transpose(1, 0, 2).reshape(E, NH * DH).astype(bf)),
            "wo": np.ascontiguousarray(
                Wo[half * NH * DH:(half + 1) * NH * DH].astype(bf)),
            "mask": mask,
        })
    return in_maps


def _ensure_ntff_hook():
    """Register the axon NTFF profile hook under antenv.axon_hooks."""
    import types
    try:
        import antenv.axon_hooks  # noqa: F401
        return
    except ImportError:
        pass
    try:
        from trn_agent_boot.trn_boot import _ntff_profile_via_ctypes
        hook = _ntff_profile_via_ctypes("/opt/axon/libaxon_pjrt.so")
    except Exception:
        hook = None
    mod = types.ModuleType("antenv.axon_hooks")
    mod.get_axon_ntff_profile_hook = lambda: hook
    mod.set_axon_ntff_profile_hook = lambda h: None
    sys.modules["antenv.axon_hooks"] = mod


def _run(inputs, trace=False):
    from concourse.bass_utils import run_bass_kernel_spmd

    if trace:
        _ensure_ntff_hook()

    x = np.asarray(inputs["x"], dtype=np.float32)
    Wq = np.asarray(inputs["Wq"], dtype=np.float32)
    Wk = np.asarray(inputs["Wk"], dtype=np.float32)
    Wv = np.asarray(inputs["Wv"], dtype=np.float32)
    Wo = np.asarray(inputs["Wo"], dtype=np.float32)
    bo = np.asarray(inputs["bo"], dtype=np.float32)

    nc = _get_nc()
    in_maps = _make_in_maps(x, Wq, Wk, Wv, Wo)
    res = run_bass_kernel_spmd(nc, in_maps, list(range(NCORES)), trace=trace)
    out = np.empty((B, S, E), dtype=np.float32)
    for b in range(B):
        out[b] = (res.results[2 * b]["out"].astype(np.float32)
                  + res.results[2 * b + 1]["out"].astype(np.float32) + bo)
    return out, res


def kernel(**inputs):
    out, _ = _run(inputs, trace=False)
    return out


# revision 39
# speedup vs baseline: 1.0443x; 1.0004x over previous
"""Trainium2 Bass kernel for causal MultiHeadAttention (B=4,S=2048,E=1024,H=16).

Sharding: 8 cores = (batch b, head-half) grid. Core c handles batch c//2 and
heads [8*(c%2), 8*(c%2)+8). Each core computes its 8 heads' attention and the
partial output projection (its 512 rows of Wo); the host sums the two fp16
partials per batch and adds the bias. ~271us/core vs the 382us v1 baseline.

Design notes (what made it fast):
  - Every matmul is a 128-partition FWL-eligible weight: scores use the
    stacked head-pair K as the stationary operand ([128 = h0 dh | h1 dh])
    against a per-head zero-padded Q moving operand (the zero half kills
    the cross-head term), so the PE runs warm 216ns/512-col MMs
    back-to-back with weight loads fully hidden. (PE row/col tiling and
    fp8 DoubleRow were measured dead ends on this hardware: non-FWL
    weight loads serialize ~100ns/MM and DoubleRow double-pumping does
    not engage, while fp8 probs cost 8x in accuracy.)
  - PV: bf16 [128,128] V weights laid out [V | ones | zeros]; one matmul
    per (head, t-tile) accumulated over the unit, partial-N on diagonal
    tiles. The ones column makes psum row 64 the softmax denominator.
  - Causality at t-tile granularity everywhere (only 62.5% of score/PV
    work is computed); the 4 diagonal 128x128 subtiles per (head, chunk)
    are fixed post-exp with one 0/1 bf16 multiply on the idle GpSimd.
  - ACT exp is the P3 pacer (~158us): emission interleaves whole P2
    projection chains and per-chunk P4 output-projection chains between
    score groups so the PE always has FWL work while ACT chews exps;
    psum pools pace scores one pr ahead of exp (PSUM: 4 banks scores,
    2 PV, 2 projections).
  - Few, large DMAs: one multi-dim descriptor per x chunk / weight pane
    (a dma_start costs ~600ns of sequencer time, so 97 startup triggers
    were ~58us of serialized pacing in earlier versions); fp16 output
    streams out per chunk; denominator reciprocal via
    reciprocal_approx_fast and a 4-trigger DRAM-bounce broadcast (the
    final unit broadcasts via a tiny fp32 outer-product matmul on the
    by-then-idle PE instead, cutting the tail's DMA round trip).
  - Warm-up matmuls during the DMA-paced first chunk keep the PE p-state
    high (cold-start MMs run 2x slow otherwise).
"""

import sys

if "/opt/trn_rl_repo" not in sys.path:
    sys.path.insert(0, "/opt/trn_rl_repo")

import numpy as np
from collections import deque
from contextlib import ExitStack

B, S, E, H = 4, 2048, 1024, 16
DH = E // H          # 64
NCORES = 8
NH = 8               # local heads per core
HP = NH // 2         # head pairs
P = 128
NE = E // P          # 8 e-tiles
NT = S // P          # 16 t-tiles
CH = 512
NCH = S // CH        # 4 q-chunks
SCALE = 1.0 / 8.0    # 1/sqrt(DH)

_CACHE = {}


def _build_nc():
    import concourse.mybir as mybir
    import concourse.tile as tile
    import concourse.bass as bass
    from concourse import bacc

    f32 = mybir.dt.float32
    f16 = mybir.dt.float16
    bf16 = mybir.dt.bfloat16
    Exp = mybir.ActivationFunctionType.Exp
    PSUM = bass.MemorySpace.PSUM

    nc = bacc.Bacc(None)
    x_d = nc.dram_tensor("x", [E, S], bf16, kind="ExternalInput")  # pre-transposed
    wq_d = nc.dram_tensor("wq", [E, NH * DH], bf16, kind="ExternalInput")
    wk_d = nc.dram_tensor("wk", [E, NH * DH], bf16, kind="ExternalInput")
    wv_d = nc.dram_tensor("wv", [E, NH * DH], bf16, kind="ExternalInput")
    wo_d = nc.dram_tensor("wo", [NH * DH, E], bf16, kind="ExternalInput")
    msk_d = nc.dram_tensor("mask", [P, P], bf16, kind="ExternalInput")
    out_d = nc.dram_tensor("out", [S, E], f16, kind="ExternalOutput")

    with ExitStack() as ctx:
        tc = ctx.enter_context(tile.TileContext(nc))
        persist = ctx.enter_context(tc.tile_pool(name="persist", bufs=1))

        qp = persist.tile([P, NH, S], bf16)            # per-head, half zero
        ks = persist.tile([P, HP, S], bf16)            # rows = stacked pair dh
        vf = persist.tile([P, NT, NH, P], bf16)        # V | ones | zeros
        msk = persist.tile([P, P], bf16)               # 0/1 causal subtile
        outTs = [persist.tile([P, S], bf16, tag=f"outT{i}", name="outT")
                 for i in range(HP)]

        # zero fills first (both engine queues are empty at t0)
        # vf ones column and zero padding (replaces v1's 4MB zz DMA)
        nc.vector.memset(vf[:, :, :, DH:DH + 1], 1.0)
        nc.vector.memset(vf[:, :, :, DH + 1:P], 0.0)
        # qp: the half of each head's 128 rows not holding Q stays zero so
        # the K=128 stacked-K score matmul drops the other head's term
        nc.gpsimd.memset(qp[DH:P, 0::2, :], 0.0)
        nc.gpsimd.memset(qp[0:DH, 1::2, :], 0.0)

        # ---- input DMAs, critical-path order ----
        # sync queue: x chunk 0 first, then the weights that feed P2
        xtp = ctx.enter_context(tc.tile_pool(name="xtp", bufs=1))
        xcs = {}

        def emit_xt(c, split=1):
            xc = xtp.tile([P, NE, CH], bf16, tag=f"xt{c % 2}", name="xt")
            step = NE // split
            for k in range(split):
                src_ap = bass.AP(
                    tensor=x_d.tensor if hasattr(x_d, 'tensor') else x_d,
                    offset=c * CH + k * step * P * S,
                    ap=[[S, P], [P * S, step], [1, CH]])
                nc.sync.dma_start(out=xc[:, k * step:(k + 1) * step, :],
                                  in_=src_ap)
            xcs[c] = xc

        emit_xt(0, split=4)
        wvall = persist.tile([P, NE, NH * DH], bf16, tag="wvall", name="wvall")
        nc.sync.dma_start(
            out=wvall,
            in_=bass.AP(
                tensor=wv_d.tensor if hasattr(wv_d, 'tensor') else wv_d,
                offset=0,
                ap=[[NH * DH, P], [P * NH * DH, NE], [1, NH * DH]]))
        emit_xt(1)
        wt2s = {}

        def emit_wt2():
            for ech in range(E // CH):
                for hp in range(HP):
                    w2 = persist.tile([P, CH], bf16, tag=f"wt2{ech}_{hp}",
                                      name="w2")
                    nc.sync.dma_start(
                        out=w2,
                        in_=wo_d[hp * P:(hp + 1) * P,
                                 ech * CH:(ech + 1) * CH])
                    wt2s[(ech, hp)] = w2

        # scalar queue (idle after startup): wq/wk per head pair + mask
        nc.scalar.dma_start(out=msk, in_=msk_d[:])
        wts = {}
        for hp in range(HP):
            for wi, wd in enumerate((wq_d, wk_d)):
                wt = persist.tile([P, NE, P], bf16, tag=f"wt{hp}{wi}",
                                  name="wt")
                nc.scalar.dma_start(
                    out=wt,
                    in_=bass.AP(
                        tensor=wd.tensor if hasattr(wd, 'tensor') else wd,
                        offset=hp * P,
                        ap=[[NH * DH, P], [P * NH * DH, NE], [1, P]]))
                wts[(hp, wi)] = wt

        # ---- pools ----
        prp = ctx.enter_context(tc.tile_pool(name="prp", bufs=2, space=PSUM))
        scp = ctx.enter_context(tc.tile_pool(name="scp", bufs=2, space=PSUM))
        pvp = ctx.enter_context(tc.tile_pool(name="pvp", bufs=2, space=PSUM))
        ptp = ctx.enter_context(tc.tile_pool(name="ptp", bufs=17))
        pop = ctx.enter_context(tc.tile_pool(name="pop", bufs=6))
        dnp = ctx.enter_context(tc.tile_pool(name="dnp", bufs=3))
        bcp = ctx.enter_context(tc.tile_pool(name="bcp", bufs=2))
        osb = ctx.enter_context(tc.tile_pool(name="osb", bufs=3))
        drp = ctx.enter_context(tc.tile_pool(name="drp", bufs=2, space="DRAM"))

        # dummy warm-up matmuls (write an unread scp column; weights = msk,
        # moving = a memset tile): keep the PE pipeline busy through the
        # DMA-paced startup so P2 chunk 0 runs at full clock
        wrm = persist.tile([P, CH], bf16, tag="wrm", name="wrm")
        nc.vector.memset(wrm, 0.0)
        lbc = persist.tile([1, DH], f32, tag="lbc", name="lbc")
        nc.vector.memset(lbc, 1.0)

        def emit_warm(n):
            ws = scp.tile([P, 2, CH], f32, tag="sp", name="sp")
            for _ in range(n):
                nc.tensor.matmul(ws[:, 0, :], msk, wrm, start=True, stop=True)

        # ---- filler work items (one whole PE chain each, ~1.8us) ----
        def gen_p2b(c, hp, wi):
            ps = prp.tile([P, CH], f32, tag="prj", name="prj")
            wt = wts[(hp, wi)]
            for et in range(NE):
                nc.tensor.matmul(ps, wt[:, et, :], xcs[c][:, et, :],
                                 start=(et == 0), stop=(et == NE - 1),
                                 skip_group_check=True)
            cs = slice(c * CH, (c + 1) * CH)
            if wi == 0:
                nc.vector.tensor_copy(
                    out=qp[0:DH, 2 * hp, cs], in_=ps[0:DH, :])
                nc.vector.tensor_copy(
                    out=qp[DH:P, 2 * hp + 1, cs], in_=ps[DH:P, :])
            else:
                nc.vector.tensor_copy(out=ks[:, hp, cs], in_=ps)
            yield

        def gen_p2a(c, sti):
            st = 4 * c + sti
            ps = prp.tile([P, CH], f32, tag="prj", name="prj")
            for et in range(NE):
                nc.tensor.matmul(
                    ps, xcs[c][:, et, sti * P:(sti + 1) * P], wvall[:, et, :],
                    start=(et == 0), stop=(et == NE - 1),
                    skip_group_check=True)
            nc.vector.tensor_copy(
                out=vf[:, st, :, 0:DH],
                in_=ps.rearrange("p (h d) -> p h d", h=NH))
            yield

        def gen_p4(c, ech, sti):
            st = 4 * c + sti
            ps = prp.tile([P, CH], f32, tag="prj", name="prj")
            for hp in range(HP):
                nc.tensor.matmul(
                    ps, outTs[hp][:, st * P:(st + 1) * P], wt2s[(ech, hp)],
                    start=(hp == 0), stop=(hp == HP - 1),
                    skip_group_check=True)
            ob = osb.tile([P, CH], f16, tag="ob", name="ob")
            nc.vector.tensor_copy(out=ob, in_=ps)
            q = nc.scalar if c == NCH - 1 else nc.sync
            q.dma_start(
                out=out_d[st * P:(st + 1) * P, ech * CH:(ech + 1) * CH],
                in_=ob)
            yield

        def p2_items(c, interleave=True):
            its = []
            for hp in range(HP):
                for wi in (0, 1):
                    its.append(gen_p2b(c, hp, wi))
                if interleave:
                    its.append(gen_p2a(c, hp))
            if not interleave:
                for sti in range(4):
                    its.append(gen_p2a(c, sti))
            return its

        # ---- P3 ----
        fin_ready = []     # chunks whose P4 can be queued
        done_units = {c: 0 for c in range(NCH)}

        def gen_pv(php, pchk, ppts, pool=None):
            """bf16 PV chains (one MM per head x t-tile) + unit tail."""
            ntv = 4 * pchk + 4
            pool = pool if pool is not None else pvp
            tg = "pv" if pool is pvp else "prj"
            pvs = {h: pool.tile([P, CH], f32, tag=tg, name="pv")
                   for h in (0, 1)}
            n = 0
            for tt in range(ntv):
                v0 = max(0, P * (tt - 4 * pchk))
                for h in (0, 1):
                    nc.tensor.matmul(
                        pvs[h][:, v0:CH],
                        vf[:, tt, 2 * php + h, :],
                        ppts[tt // 2][h][:, tt % 2, v0:CH],
                        start=(tt == 0), stop=(tt == ntv - 1),
                        skip_group_check=True)
                    n += 1
                    if n % 4 == 0:
                        yield
            # tail: numerators -> po; per-head reciprocal denominator,
            # DRAM-bounce stride-0 broadcast (4 DMA triggers total), outT
            # scale on gpsimd
            po = pop.tile([P, CH], bf16, tag="po", name="po")
            cs = slice(pchk * CH, (pchk + 1) * CH)
            final = php == HP - 1 and pchk == NCH - 1
            if final:
                # tail path: broadcast 1/den via a tiny fp32 outer-product
                # matmul on the (idle) PE instead of the DRAM bounce
                bcps = prp.tile([P, CH], f32, tag="prj", name="prj")
                for h in (0, 1):
                    nc.vector.tensor_copy(
                        out=po[h * DH:(h + 1) * DH, :], in_=pvs[h][0:DH, :])
                    den = dnp.tile([1, CH], f32, tag="den", name="den")
                    nc.vector.tensor_copy(out=den, in_=pvs[h][DH:DH + 1, :])
                    rd = dnp.tile([1, CH], f32, tag="rd", name="rd")
                    nc.vector.reciprocal_approx_fast(out=rd, in_=den)
                    nc.tensor.matmul(
                        bcps[h * DH:(h + 1) * DH, :], lbc, rd,
                        start=True, stop=True, skip_group_check=True)
                nc.vector.tensor_mul(outTs[php][:, cs], po, bcps)
            else:
                dd = drp.tile([2, CH], f32, tag="dd", name="dd")
                bc = bcp.tile([P, CH], f32, tag="bc", name="bc")
                for h in (0, 1):
                    nc.vector.tensor_copy(
                        out=po[h * DH:(h + 1) * DH, :], in_=pvs[h][0:DH, :])
                    den = dnp.tile([1, CH], f32, tag="den", name="den")
                    nc.vector.tensor_copy(out=den, in_=pvs[h][DH:DH + 1, :])
                    rd = dnp.tile([1, CH], f32, tag="rd", name="rd")
                    nc.vector.reciprocal_approx_fast(out=rd, in_=den)
                    nc.sync.dma_start(out=dd[h:h + 1, :], in_=rd)
                for h in (0, 1):
                    row = dd[h:h + 1, :]
                    bsrc = bass.AP(
                        tensor=row.tensor, offset=row.offset,
                        ap=[[0, DH]] + list(row.ap[1:]))
                    nc.sync.dma_start(
                        out=bc[h * DH:(h + 1) * DH, :], in_=bsrc)
                nc.gpsimd.tensor_mul(outTs[php][:, cs], po, bc)
            done_units[pchk] += 1
            if done_units[pchk] == HP:
                fin_ready.append(pchk)

        fill_p2 = deque()
        fill_p4 = deque()
        pvgen = None

        def drain_one(q):
            while q:
                try:
                    next(q[0])
                    return 1
                except StopIteration:
                    q.popleft()
            return 0

        def drain_fill(k):
            n = 0
            for i in range(k):
                got = drain_one(fill_p2 if i % 2 == 0 else fill_p4)
                if not got:
                    got = drain_one(fill_p4 if i % 2 == 0 else fill_p2)
                n += got
                if not got:
                    break
            return n

        def emit_unit(hp, chk, pv_steps, fill_steps, selfpv=False):
            nonlocal pvgen
            nprs = 2 * chk + 2
            pts = []
            own = None
            for pr in range(nprs):
                sps = {h: scp.tile([P, 2, CH], f32, tag="sp", name="sp")
                       for h in (0, 1)}
                for j in (0, 1):
                    tt = 2 * pr + j
                    v0 = max(0, P * (tt - 4 * chk))
                    for h in (0, 1):
                        nc.tensor.matmul(
                            sps[h][:, j, v0:CH],
                            ks[:, hp, tt * P:(tt + 1) * P],
                            qp[:, 2 * hp + h,
                               chk * CH + v0:(chk + 1) * CH],
                            start=True, stop=True)
                pt = {h: ptp.tile([P, 2, CH], bf16, tag="pt", name="pt")
                      for h in (0, 1)}
                diag = pr >= 2 * chk
                for h in (0, 1):
                    if diag:
                        v00 = P * (2 * pr - 4 * chk)
                        nc.scalar.activation(
                            out=pt[h][:, :, v00:CH], in_=sps[h][:, :, v00:CH],
                            func=Exp, scale=SCALE)
                        for j in (0, 1):
                            va = v00 + j * P
                            nc.gpsimd.tensor_mul(
                                pt[h][:, j, va:va + P],
                                pt[h][:, j, va:va + P], msk)
                    else:
                        nc.scalar.activation(
                            out=pt[h][:, :, :], in_=sps[h][:, :, :],
                            func=Exp, scale=SCALE)
                pts.append(pt)
                # filler: pending-unit PV matmuls + P2/P4 chains
                if pvgen is not None:
                    for _ in range(pv_steps):
                        try:
                            next(pvgen)
                        except StopIteration:
                            pvgen = None
                            break
                if selfpv and pvgen is None:
                    if own is None:
                        own = gen_pv(hp, chk, pts)
                    try:
                        next(own)
                    except StopIteration:
                        own = None
                drain_fill(fill_steps)
            if selfpv and own is not None:
                for _ in own:
                    pass
            return pts, (own is None and selfpv)

        # ---- main emission ----
        # head: P2 of chunk 0 (run to completion; first scores follow)
        for gi, g in enumerate(p2_items(0, interleave=False)):
            if gi < 4:
                emit_warm(2)
            for _ in g:
                pass

        units = [(hp, chk) for chk in range(NCH) for hp in range(HP)]
        pend = None
        for hp, chk in units:
            if hp == 0 and chk + 1 < NCH:
                if chk + 2 < NCH:
                    emit_xt(chk + 2)
                if chk == 0:
                    emit_wt2()
                for g in p2_items(chk + 1):
                    fill_p2.append(g)
            # previous unit's PV drains across this unit's prs (lag-1;
            # sp-slot pacing guarantees its exps have completed)
            nprs = 2 * chk + 2
            if pend is not None:
                pvgen = gen_pv(pend[0], pend[1], pend[2])
                pntv = 4 * pend[1] + 4
                pv_steps = (pntv // 2 + nprs - 1) // nprs + 1
            else:
                pv_steps = 0
            last = False  # zero-lag self-PV measured slightly worse
            pts, pv_done = emit_unit(hp, chk, pv_steps,
                                     2 if chk < 2 else 1, selfpv=last)
            if pvgen is not None:
                for _ in pvgen:
                    pass
                pvgen = None
            pend = None if pv_done else (hp, chk, pts)
            while fin_ready:
                c = fin_ready.pop(0)
                for ech in range(E // CH):
                    for sti in range(4):
                        fill_p4.append(gen_p4(c, ech, sti))
            # chunk boundary: next chunk's P2 must be fully emitted
            if hp == HP - 1:
                while drain_one(fill_p2):
                    pass

        # tail: last unit's PV (if not already emitted), finalize, last P4
        if pend is not None:
            for _ in gen_pv(pend[0], pend[1], pend[2], pool=prp):
                pass
        while fin_ready:
            c = fin_ready.pop(0)
            for ech in range(E // CH):
                for sti in range(4):
                    fill_p4.append(gen_p4(c, ech, sti))
        while drain_fill(64):
            pass

    nc.finalize()
    return nc


def _get_nc():
    if "nc" not in _CACHE:
        _CACHE["nc"] = _build_nc()
    return _CACHE["nc"]


def _make_in_maps(x, Wq, Wk, Wv, Wo):
    import ml_dtypes

    bf = ml_dtypes.bfloat16
    # multiplicative causal mask for a diagonal 128x128 subtile
    pcol = np.arange(P)[:, None]
    frow = np.arange(P)[None, :]
    mask = (pcol <= frow).astype(bf)
    in_maps = []
    for c in range(NCORES):
        b, half = divmod(c, 2)
        hs = slice(half * NH, (half + 1) * NH)
        in_maps.append({
            "x": np.ascontiguousarray(x[b].T.astype(bf)),
            "wq": np.ascontiguousarray(
                Wq[hs].transpose(1, 0, 2).reshape(E, NH * DH).astype(bf)),
            "wk": np.ascontiguousarray(
                Wk[hs].transpose(1, 0, 2).reshape(E, NH * DH).astype(bf)),
            "wv": np.ascontiguousarray(
                Wv[hs].transpose(1, 0, 2).reshape(E, NH * DH).astype(bf)),
            "wo": np.ascontiguousarray(
                Wo[half * NH * DH:(half + 1) * NH * DH].astype(bf)),
            "mask": mask,
        })
    return in_maps


def _ensure_ntff_hook():
    """Register the axon NTFF profile hook under antenv.axon_hooks."""
    import types
    try:
        import antenv.axon_hooks  # noqa: F401
        return
    except ImportError:
        pass
    try:
        from trn_agent_boot.trn_boot import _ntff_profile_via_ctypes
        hook = _ntff_profile_via_ctypes("/opt/axon/libaxon_pjrt.so")
    except Exception:
        hook = None
    mod = types.ModuleType("antenv.axon_hooks")
    mod.get_axon_ntff_profile_hook = lambda: hook
    mod.set_axon_ntff_profile_hook = lambda h: None
    sys.modules["antenv.axon_hooks"] = mod


def _run(inputs, trace=False):
    from concourse.bass_utils import run_bass_kernel_spmd

    if trace:
        _ensure_ntff_hook()

    x = np.asarray(inputs["x"], dtype=np.float32)
    Wq = np.asarray(inputs["Wq"], dtype=np.float32)
    Wk = np.asarray(inputs["Wk"], dtype=np.float32)
    Wv = np.asarray(inputs["Wv"], dtype=np.float32)
    Wo = np.asarray(inputs["Wo"], dtype=np.float32)
    bo = np.asarray(inputs["bo"], dtype=np.float32)

    nc = _get_nc()
    in_maps = _make_in_maps(x, Wq, Wk, Wv, Wo)
    res = run_bass_kernel_spmd(nc, in_maps, list(range(NCORES)), trace=trace)
    out = np.empty((B, S, E), dtype=np.float32)
    for b in range(B):
        out[b] = (res.results[2 * b]["out"].astype(np.float32)
                  + res.results[2 * b + 1]["out"].astype(np.float32) + bo)
    return out, res


def kernel(**inputs):
    out, _ = _run(inputs, trace=False)
    return out
